# revision 1
# baseline (speedup 1.0000x reference)
"""Trainium2 Bass kernel for nn_CLinear_6768868459230.

Context-conditioned block-autoregressive linear layer (MAF-style):
  wdir = c @ Wd + bd                      [B, O, I]
  w    = exp(wdir)*mask_diag + wdir*mask_lower
  sqn  = sum(w^2, axis=I)
  y    = (w / sqrt(sqn) * exp(wamp)) @ xv + bias
  logdet = logsumexp over diag block of (wdir - 0.5 log sqn + wamp + xl)

Sharding: tensor-parallel over the O=512 output rows (the 262144-wide Wd
matmul dominates). Each of the 8 cores owns 8 of the 64 channels, chosen
as pairs {k, 15-k, 16+k, 31-k, ...} so the block-triangular work (rows of
channel ch touch only ch*8 input columns) is identical on every core —
required anyway because all cores execute one shared program.

Only the strictly-lower + diagonal columns of Wd are shipped/loaded
(the rest are masked to zero by the reference), roughly halving traffic.
Per-row lower widths are zero-padded up to a per-slot maximum W(j)=64j+56
so the instruction stream is core-independent; zero padding is exact
(contributes 0 to both sums).

On-device per core (per 128-sample batch chunk):
  TensorE : wdir lower segments + diag block via cT.T @ Wd (float32r),
            with bd added as K=1 ones-outer-product accumulating matmuls
  ScalarE : per-row sum(t^2) via activation(Square, accum_out)
  VectorE : per-row sum(t * xv) via tensor_tensor_reduce
  diag    : batched 512-wide: exp(td), exp(2 td), products with xv and
            exp(xl), segmented tensor_reduce over fin=8
  logdet  = wamp - 0.5*ln(sqn) + ln(sum_f exp(td + xl))  (no max-trick
            needed: |td + xl| <~ 8 at these scales)
"""

import numpy as np

NCH, FIN, FOUT, CDIM, B = 64, 8, 8, 128, 256
I = NCH * FIN
O = NCH * FOUT
NCORES = 8
NLOC = 64  # output rows per core
BCH = 128  # batch chunk (SBUF partitions)

# per-slot padded lower width and rows-per-matmul grouping
W_OF_J = [64 * j + 56 for j in range(8)]
G_OF_J = [8, 4, 2, 2, 1, 1, 1, 1]  # rows per matmul so N = G*W <= 512
BASE_J = [8 * sum(W_OF_J[:j]) for j in range(8)]
TOTW = 8 * sum(W_OF_J)  # 17920


def _channels(k):
    return [k, 15 - k, 16 + k, 31 - k, 32 + k, 47 - k, 48 + k, 63 - k]


_NC = None
_REPEAT = 1  # bench knob: replicate compute body


def _build_nc():
    import concourse.bacc as bacc
    import concourse.tile as tile
    from concourse import mybir

    f32 = mybir.dt.float32
    f32r = mybir.dt.float32r
    bf16 = mybir.dt.bfloat16
    AF = mybir.ActivationFunctionType
    ALU = mybir.AluOpType

    nc = bacc.Bacc(None, target_bir_lowering=False)

    d_cT = nc.dram_tensor("ct", [CDIM, B], bf16, kind="ExternalInput")
    d_cTf = nc.dram_tensor("ctf", [CDIM, B], f32r, kind="ExternalInput")
    d_wdl = nc.dram_tensor("wdl", [CDIM, TOTW], bf16, kind="ExternalInput")
    d_wdd = nc.dram_tensor("wdd", [CDIM, NLOC * FIN], f32r, kind="ExternalInput")
    d_bdd = nc.dram_tensor("bdd", [1, NLOC * FIN], f32r, kind="ExternalInput")
    d_ew = nc.dram_tensor("ew", [CDIM, 3 * NLOC], bf16, kind="ExternalInput")
    d_eb = nc.dram_tensor("eb", [1, 3 * NLOC], bf16, kind="ExternalInput")
    d_xvT = nc.dram_tensor("xvt", [I, B], bf16, kind="ExternalInput")
    d_bdm = nc.dram_tensor("bdm", [I, NLOC], bf16, kind="ExternalInput")
    d_xv = nc.dram_tensor("xv", [B, I], f32, kind="ExternalInput")
    d_xvd = nc.dram_tensor("xvd", [B, NLOC * FIN], f32, kind="ExternalInput")
    d_xle = nc.dram_tensor("xle", [B, NLOC * FIN], f32, kind="ExternalInput")
    d_out = nc.dram_tensor("out", [B, NLOC, 2], f32, kind="ExternalOutput")

    with tile.TileContext(nc) as tc:
        with (
            tc.tile_pool(name="consts", bufs=1) as consts,
            tc.tile_pool(name="scr", bufs=3) as scr,
            tc.tile_pool(name="accs", bufs=2) as accs,
            tc.tile_pool(name="segp", bufs=6, space="PSUM") as segp,
            tc.tile_pool(name="miscp", bufs=1, space="PSUM") as miscp,
            tc.tile_pool(name="extp", bufs=1, space="PSUM") as extp,
        ):
            # ---- constants / weights ----
            # sync queue: ct + xv first (gate the matmuls / dot products),
            # then even weight slots. scalar queue: small matmul operands,
            # then odd weight slots. The two HW DGEs stream in parallel.
            ct_sb = consts.tile([CDIM, B], bf16)
            nc.sync.dma_start(out=ct_sb, in_=d_cT[:, :])
            ctf_sb = consts.tile([CDIM, B], f32r)
            nc.sync.dma_start(out=ctf_sb, in_=d_cTf[:, :])
            onesf_sb = consts.tile([1, BCH], f32r)
            nc.vector.memset(onesf_sb.bitcast(mybir.dt.uint32), 0x3F800000)
            ones_sb = consts.tile([1, BCH], bf16)
            nc.vector.memset(ones_sb, 1.0)
            xv_sb, xvd_sb, xle_sb = [], [], []
            for bc in range(2):
                b0 = bc * BCH
                t = consts.tile([BCH, I], f32, name=f"xv{bc}", tag=f"xv{bc}")
                nc.sync.dma_start(out=t, in_=d_xv[b0 : b0 + BCH, :])
                xv_sb.append(t)
            wdd_sb = consts.tile([CDIM, NLOC * FIN], f32r)
            nc.scalar.dma_start(out=wdd_sb, in_=d_wdd[:, :])
            bdd_sb = consts.tile([1, NLOC * FIN], f32r)
            nc.scalar.dma_start(out=bdd_sb, in_=d_bdd[:, :])
            ew_sb = consts.tile([CDIM, 3 * NLOC], bf16)
            nc.scalar.dma_start(out=ew_sb, in_=d_ew[:, :])
            eb_sb = consts.tile([1, 3 * NLOC], bf16)
            nc.scalar.dma_start(out=eb_sb, in_=d_eb[:, :])
            xvt_sb = consts.tile([CDIM, 4, B], bf16)
            for kc in range(4):
                nc.scalar.dma_start(
                    out=xvt_sb[:, kc, :], in_=d_xvT[kc * 128 : (kc + 1) * 128, :]
                )
            bdm_sb = consts.tile([CDIM, 4, NLOC], bf16)
            for kc in range(4):
                nc.scalar.dma_start(
                    out=bdm_sb[:, kc, :], in_=d_bdm[kc * 128 : (kc + 1) * 128, :]
                )
            wdl_sb = [None] * 8
            ENG_OF_J = ["sync", "scalar", "sync", "scalar",
                        "sync", "gps", "scalar", "gps"]
            for j in (0, 1, 2, 3, 4, 5, 6, 7):
                w = W_OF_J[j]
                t = consts.tile([CDIM, 8 * w], bf16, name=f"wdl{j}", tag=f"wdl{j}")
                eng = {"sync": nc.sync, "scalar": nc.scalar, "gps": nc.gpsimd}[
                    ENG_OF_J[j]
                ]
                eng.dma_start(out=t, in_=d_wdl[:, BASE_J[j] : BASE_J[j] + 8 * w])
                wdl_sb[j] = t
            for bc in range(2):
                b0 = bc * BCH
                t = consts.tile([BCH, NLOC * FIN], f32, name=f"xvd{bc}", tag=f"xvd{bc}")
                nc.sync.dma_start(out=t, in_=d_xvd[b0 : b0 + BCH, :])
                xvd_sb.append(t)
                t = consts.tile([BCH, NLOC * FIN], f32, name=f"xle{bc}", tag=f"xle{bc}")
                nc.sync.dma_start(out=t, in_=d_xle[b0 : b0 + BCH, :])
                xle_sb.append(t)

            for _rep in range(_REPEAT):
                BATCH_SQ = (0, 1)  # slots whose squares are segment-batched
                st = {}
                # ---- phase A: matmuls + squares + dot products ----
                for bc in range(2):
                    b0 = bc * BCH
                    lhs = ct_sb[:, b0 : b0 + BCH]
                    xv_b = xv_sb[bc]

                    # extras: wamp | bias | 2*c@g | dotbd
                    pex = extp.tile([BCH, 4 * NLOC], f32, name="pex", tag="pex")
                    nc.tensor.matmul(
                        pex[:, : 3 * NLOC], lhs, ew_sb, start=True, stop=False
                    )
                    nc.tensor.matmul(
                        pex[:, : 3 * NLOC], ones_sb, eb_sb, start=False, stop=True
                    )
                    for kc in range(4):
                        nc.tensor.matmul(
                            pex[:, 3 * NLOC :],
                            xvt_sb[:, kc, b0 : b0 + BCH],
                            bdm_sb[:, kc, :],
                            start=(kc == 0),
                            stop=(kc == 3),
                        )

                    # diag block matmul (elementwise work deferred to phase B)
                    pdg = miscp.tile([BCH, NLOC * FIN], f32, name="pdg", tag="pdg")
                    nc.tensor.matmul(pdg, ctf_sb[:, b0 : b0 + BCH], wdd_sb, start=True, stop=False)
                    nc.tensor.matmul(pdg, onesf_sb, bdd_sb, start=False, stop=True)

                    SQL = accs.tile([BCH, NLOC], f32, name="SQL", tag="SQL")
                    DOTL = accs.tile([BCH, NLOC], f32, name="DOTL", tag="DOTL")
                    sq_pend = []
                    for j in range(8):
                        w, g = W_OF_J[j], G_OF_J[j]
                        prodj = scr.tile(
                            [BCH, 8 * 504], f32, name="prodj", tag="prodj", bufs=2
                        )
                        sqbj = None
                        if j in BATCH_SQ:
                            sqbj = scr.tile(
                                [BCH, 8 * 120], f32, name="sqbj", tag="sqbj", bufs=2
                            )
                        for s in range(8 // g):
                            r0 = j * 8 + s * g
                            n = g * w
                            ps = segp.tile([BCH, 512], f32, name="ps", tag="ps")
                            nc.tensor.matmul(
                                ps[:, :n],
                                lhs,
                                wdl_sb[j][:, s * n : (s + 1) * n],
                                start=True,
                                stop=True,
                            )
                            if j in BATCH_SQ:
                                nc.scalar.activation(
                                    out=sqbj[:, s * n : (s + 1) * n],
                                    in_=ps[:, :n],
                                    func=AF.Square,
                                )
                            else:
                                for q in range(g):
                                    r = r0 + q
                                    a = q * w
                                    sS = scr.tile(
                                        [BCH, 504], f32, name="sS", tag="sS"
                                    )
                                    nc.scalar.activation(
                                        out=sS[:, :w],
                                        in_=ps[:, a : a + w],
                                        func=AF.Square,
                                        accum_out=SQL[:, r : r + 1],
                                    )
                            # t * xv for all g rows (xv broadcast over rows)
                            if g == 1:
                                nc.vector.tensor_mul(
                                    prodj[:, s * n : (s + 1) * n],
                                    ps[:, :n],
                                    xv_b[:, :w],
                                )
                            else:
                                nc.vector.tensor_mul(
                                    prodj[:, s * n : (s + 1) * n].rearrange(
                                        "p (g w) -> p g w", w=w
                                    ),
                                    ps[:, :n].rearrange("p (g w) -> p g w", w=w),
                                    xv_b[:, :w].unsqueeze(1).broadcast_to(
                                        [BCH, g, w]
                                    ),
                                )
                        nc.vector.tensor_reduce(
                            out=DOTL[:, j * 8 : (j + 1) * 8],
                            in_=prodj[:, : 8 * w].rearrange("p (r w) -> p r w", w=w),
                            axis=mybir.AxisListType.X,
                            op=ALU.add,
                        )
                        if j in BATCH_SQ:
                            sq_pend.append((j, w, sqbj))
                    for j, w, sqbj in sq_pend:
                        nc.vector.tensor_reduce(
                            out=SQL[:, j * 8 : (j + 1) * 8],
                            in_=sqbj[:, : 8 * w].rearrange("p (r w) -> p r w", w=w),
                            axis=mybir.AxisListType.X,
                            op=ALU.add,
                        )
                    st[bc] = dict(pex=pex, pdg=pdg, SQL=SQL, DOTL=DOTL)

                # ---- phase B: diag elementwise (all Exp — one table set) ----
                for bc in range(2):
                    s_ = st[bc]
                    pdg = s_["pdg"]
                    expd = scr.tile(
                        [BCH, NLOC * FIN], f32, name="expd", tag="expd", bufs=2
                    )
                    nc.scalar.activation(out=expd, in_=pdg, func=AF.Exp)
                    sq2 = scr.tile(
                        [BCH, NLOC * FIN], f32, name="sq2", tag="sq2", bufs=2
                    )
                    nc.scalar.activation(out=sq2, in_=pdg, func=AF.Exp, scale=2.0)
                    SQD = accs.tile([BCH, NLOC], f32, name="SQD", tag="SQD")
                    nc.vector.tensor_reduce(
                        out=SQD,
                        in_=sq2.rearrange("p (r f) -> p r f", f=FIN),
                        axis=mybir.AxisListType.X,
                        op=ALU.add,
                    )
                    prd = scr.tile(
                        [BCH, NLOC * FIN], f32, name="prd", tag="prd", bufs=2
                    )
                    nc.gpsimd.tensor_mul(prd, expd, xvd_sb[bc])
                    DOTD = accs.tile([BCH, NLOC], f32, name="DOTD", tag="DOTD")
                    nc.vector.tensor_reduce(
                        out=DOTD,
                        in_=prd.rearrange("p (r f) -> p r f", f=FIN),
                        axis=mybir.AxisListType.X,
                        op=ALU.add,
                    )
                    prl = scr.tile(
                        [BCH, NLOC * FIN], f32, name="prl", tag="prl", bufs=2
                    )
                    nc.gpsimd.tensor_mul(prl, expd, xle_sb[bc])
                    LDS = accs.tile([BCH, NLOC], f32, name="LDS", tag="LDS")
                    nc.vector.tensor_reduce(
                        out=LDS,
                        in_=prl.rearrange("p (r f) -> p r f", f=FIN),
                        axis=mybir.AxisListType.X,
                        op=ALU.add,
                    )
                    s_.update(SQD=SQD, DOTD=DOTD, LDS=LDS)

                # ---- phase C: assembly. ScalarE order: Ln*4, Copy*2, Exp*2 ----
                for bc in range(2):
                    s_ = st[bc]
                    sqn = accs.tile([BCH, NLOC], f32, name="sqn", tag="sqn")
                    nc.vector.tensor_add(sqn, s_["SQL"], s_["SQD"])
                    nc.vector.tensor_add(
                        sqn, sqn, s_["pex"][:, 2 * NLOC : 3 * NLOC]
                    )
                    dot = accs.tile([BCH, NLOC], f32, name="dot", tag="dot")
                    nc.vector.tensor_add(dot, s_["DOTL"], s_["DOTD"])
                    nc.vector.tensor_add(dot, dot, s_["pex"][:, 3 * NLOC :])
                    s_.update(sqn=sqn, dot=dot)
                for bc in range(2):
                    s_ = st[bc]
                    l1 = accs.tile([BCH, NLOC], f32, name="l1", tag="l1")
                    nc.scalar.activation(out=l1, in_=s_["sqn"], func=AF.Ln)
                    l2 = accs.tile([BCH, NLOC], f32, name="l2", tag="l2")
                    nc.scalar.activation(out=l2, in_=s_["LDS"], func=AF.Ln)
                    s_.update(l1=l1, l2=l2)
                for bc in range(2):
                    s_ = st[bc]
                    m1 = accs.tile([BCH, NLOC], f32, name="m1", tag="m1")
                    nc.scalar.mul(m1, s_["l1"], -0.5)
                    u = accs.tile([BCH, NLOC], f32, name="u", tag="u")
                    nc.vector.tensor_add(u, s_["pex"][:, :NLOC], m1)
                    s_.update(u=u)
                for bc in range(2):
                    s_ = st[bc]
                    sc = accs.tile([BCH, NLOC], f32, name="sc", tag="sc")
                    nc.scalar.activation(out=sc, in_=s_["u"], func=AF.Exp)
                    s_.update(sc=sc)
                for bc in range(2):
                    b0 = bc * BCH
                    s_ = st[bc]
                    yv = accs.tile([BCH, NLOC], f32, name="yv", tag="yv")
                    nc.vector.tensor_mul(yv, s_["dot"], s_["sc"])
                    yb = accs.tile([BCH, NLOC], f32, name="yb", tag="yb")
                    nc.vector.tensor_add(yb, yv, s_["pex"][:, NLOC : 2 * NLOC])
                    ld = accs.tile([BCH, NLOC], f32, name="ld", tag="ld")
                    nc.vector.tensor_add(ld, s_["u"], s_["l2"])
                    ob = accs.tile([BCH, NLOC, 2], f32, name="ob", tag="ob")
                    nc.gpsimd.tensor_copy(out=ob[:, :, 0], in_=yb)
                    nc.gpsimd.tensor_copy(out=ob[:, :, 1], in_=ld)
                    nc.sync.dma_start(out=d_out[b0 : b0 + BCH, :, :], in_=ob)

    nc.compile()
    return nc


def _host_prep(x, c, Wd, bd, Wa, ba, Wb, bb):
    """Build the 8 per-core input maps."""
    import ml_dtypes

    bf = ml_dtypes.bfloat16
    x = np.ascontiguousarray(x, dtype=np.float32)
    c = np.ascontiguousarray(c, dtype=np.float32)
    Wd5 = np.ascontiguousarray(Wd, dtype=np.float32).reshape(CDIM, NCH, FOUT, NCH, FIN)
    bd4 = np.ascontiguousarray(bd, dtype=np.float32).reshape(NCH, FOUT, NCH, FIN)
    Wa = np.ascontiguousarray(Wa, dtype=np.float32)
    Wb = np.ascontiguousarray(Wb, dtype=np.float32)
    ba = np.ascontiguousarray(ba, dtype=np.float32)
    bb = np.ascontiguousarray(bb, dtype=np.float32)

    cT = np.ascontiguousarray(c.T)
    xv = np.ascontiguousarray(x[:, :, 0])
    xl = np.ascontiguousarray(x[:, :, 1])
    xvT = np.ascontiguousarray(xv.T)

    in_maps = []
    for k in range(NCORES):
        chs = _channels(k)
        wdl = np.zeros((CDIM, TOTW), dtype=np.float32)
        wdd = np.empty((CDIM, NLOC * FIN), dtype=np.float32)
        bdd = np.empty((1, NLOC * FIN), dtype=np.float32)
        ew = np.zeros((CDIM, 3 * NLOC), dtype=np.float32)
        eb = np.zeros((1, 3 * NLOC), dtype=np.float32)
        bdm = np.zeros((I, NLOC), dtype=np.float32)
        xvd = np.empty((B, NLOC * FIN), dtype=np.float32)
        xled = np.empty((B, NLOC * FIN), dtype=np.float32)
        for j, ch in enumerate(chs):
            w = W_OF_J[j]
            for fo in range(FOUT):
                lo = BASE_J[j] + fo * w
                wdl[:, lo : lo + ch * FIN] = Wd5[:, ch, fo, :ch, :].reshape(CDIM, -1)
                r = j * FOUT + fo
                wdd[:, r * FIN : (r + 1) * FIN] = Wd5[:, ch, fo, ch, :]
                bdd[0, r * FIN : (r + 1) * FIN] = bd4[ch, fo, ch, :]
                bd_low = bd4[ch, fo, :ch, :].reshape(-1)  # true lower bd row
                bdm[: ch * FIN, r] = bd_low
                # sqn cross term 2*(c@g) and constant sum(bd^2)
                wl = Wd5[:, ch, fo, :ch, :].reshape(CDIM, -1)
                ew[:, 2 * NLOC + r] = 2.0 * (wl @ bd_low)
                eb[0, 2 * NLOC + r] = np.dot(bd_low, bd_low)
            rows = slice(ch * FOUT, (ch + 1) * FOUT)
            ew[:, j * FOUT : (j + 1) * FOUT] = Wa[:, rows]
            ew[:, NLOC + j * FOUT : NLOC + (j + 1) * FOUT] = Wb[:, rows]
            eb[0, j * FOUT : (j + 1) * FOUT] = ba[rows]
            eb[0, NLOC + j * FOUT : NLOC + (j + 1) * FOUT] = bb[rows]
        for r in range(NLOC):
            j = r // FOUT
            ch = chs[j]
            xvd[:, r * FIN : (r + 1) * FIN] = xv[:, ch * FIN : (ch + 1) * FIN]
            xled[:, r * FIN : (r + 1) * FIN] = xl[:, ch * FIN : (ch + 1) * FIN]
        np.exp(xled, out=xled)

        in_maps.append(
            {
                "ct": cT.astype(bf),
                "ctf": cT,
                "wdl": wdl.astype(bf),
                "wdd": wdd,
                "bdd": bdd,
                "ew": ew.astype(bf),
                "eb": eb.astype(bf),
                "xv": xv,
                "xvt": xvT.astype(bf),
                "bdm": bdm.astype(bf),
                "xvd": xvd,
                "xle": xled,
            }
        )
    return in_maps


def kernel(x, c, Wd, bd, Wa, ba, Wb, bb, _trace=False, _tmpdir=None):
    global _NC
    from concourse.bass_utils import run_bass_kernel_spmd

    if _NC is None:
        _NC = _build_nc()
    in_maps = _host_prep(x, c, Wd, bd, Wa, ba, Wb, bb)
    res = run_bass_kernel_spmd(
        _NC, in_maps, core_ids=list(range(NCORES)), trace=_trace, tmpdir=_tmpdir
    )

    out = np.empty((B, O, 2), dtype=np.float32)
    for k in range(NCORES):
        ok = res.results[k]["out"]
        for j, ch in enumerate(_channels(k)):
            out[:, ch * FOUT : (ch + 1) * FOUT, :] = ok[:, j * FOUT : (j + 1) * FOUT, :]
    if _trace:
        return out, res
    return out



# revision 6
# speedup vs baseline: 1.0978x; 1.0978x over previous
"""Trainium2 Bass kernel for nn_CLinear_6768868459230.

Context-conditioned block-autoregressive linear layer (MAF-style):
  wdir = c @ Wd + bd                      [B, O, I]
  w    = exp(wdir)*mask_diag + wdir*mask_lower
  sqn  = sum(w^2, axis=I)
  y    = (w / sqrt(sqn) * exp(wamp)) @ xv + bias
  logdet = logsumexp over diag block of (wdir - 0.5 log sqn + wamp + xl)

Sharding: tensor-parallel over the O=512 output rows. Each of the 8 cores
owns 8 of the 64 channels, chosen as pairs {k, 15-k, 16+k, 31-k, ...} so
the block-triangular work is nearly identical on every core (all cores
execute one shared program; per-slot shapes padded to the slot max).

v2 redesign (vs the t-form baseline): avoid materializing the [B, ~17k]
lower products t = c @ Wd_lower entirely (its elementwise square/dot
reductions were the Vector/Scalar bottleneck).  Instead:

  dot_lower[b,o] = c_b^T (W_o x_b)   computed as
     R[b,(o,k)] = sum_i W[k,o,i] x[b,i]   (TensorE, contraction over i
        in chunks of <=128, K = true i-extent so no zero padding cost)
     dot = sum_k c[b,k] * R[b,(o,k)]      (one bf16 multiply at DVE 2x
        + a bf16 add-tree; only 128 terms per row instead of w_o<=504)

  sqn_lower[b,o] = |C_o^T c_b|^2 + 2(G_o bd_o)?c_b + |bd_o|^2  where
     C_o = G_o itself for w_o <= 128, else chol(G_o G_o^T) [128x128]
     (host-side, weights-only transform).  ScalarE squares the <=128-wide
     s = C^T c, a bf16 add-tree reduces.  This caps the square work at
     128 terms per row too.

Diag block (exp / logsumexp over FIN=8) and the small context nets are
unchanged from the baseline, but run in bf16 instead of f32r.
"""

import numpy as np

NCH, FIN, FOUT, CDIM, B = 64, 8, 8, 128, 256
I = NCH * FIN
O = NCH * FOUT
NCORES = 8
NLOC = 64  # output rows per core
BCH = 128  # batch chunk (SBUF partitions)

# per-slot i-chunk extents (max over cores of w=8*ch for that slot)
EXT_OF_J = [
    [56],
    [120],
    [128, 56],
    [128, 120],
    [128, 128, 56],
    [128, 128, 120],
    [128, 128, 128, 56],
    [128, 128, 128, 120],
]
SLAB_ROWS = [e for j in range(8) for e in EXT_OF_J[j]]  # 20 slabs
SLAB_BASE = np.concatenate([[0], np.cumsum(SLAB_ROWS)]).tolist()
TOT_SLAB = SLAB_BASE[-1]  # 2240


def _channels(k):
    return [k, 15 - k, 16 + k, 31 - k, 32 + k, 47 - k, 48 + k, 63 - k]


_NC = None


def _build_nc():
    import concourse.bacc as bacc
    import concourse.tile as tile
    from concourse import mybir

    f32 = mybir.dt.float32
    bf16 = mybir.dt.bfloat16
    AF = mybir.ActivationFunctionType
    ALU = mybir.AluOpType

    nc = bacc.Bacc(None, target_bir_lowering=False)

    d_ct = nc.dram_tensor("ct", [CDIM, B], bf16, kind="ExternalInput")
    d_cb = nc.dram_tensor("cb", [B, CDIM], bf16, kind="ExternalInput")
    d_xvT = nc.dram_tensor("xvt", [I, B], bf16, kind="ExternalInput")
    d_wsl = nc.dram_tensor("wsl", [TOT_SLAB, 8 * 128], bf16, kind="ExternalInput")
    d_csl = nc.dram_tensor("csl", [CDIM, NLOC * 128], bf16, kind="ExternalInput")
    d_wdd = nc.dram_tensor("wdd", [CDIM, NLOC * FIN], bf16, kind="ExternalInput")
    d_bdd = nc.dram_tensor("bdd", [1, NLOC * FIN], bf16, kind="ExternalInput")
    d_ew = nc.dram_tensor("ew", [CDIM, 3 * NLOC], bf16, kind="ExternalInput")
    d_eb = nc.dram_tensor("eb", [1, 3 * NLOC], bf16, kind="ExternalInput")
    d_bdm = nc.dram_tensor("bdm", [I, NLOC], bf16, kind="ExternalInput")
    d_xvd = nc.dram_tensor("xvd", [B, NLOC * FIN], bf16, kind="ExternalInput")
    d_xle = nc.dram_tensor("xle", [B, NLOC * FIN], bf16, kind="ExternalInput")
    d_out = nc.dram_tensor("out", [B, NLOC, 2], f32, kind="ExternalOutput")

    with tile.TileContext(nc) as tc:
        with (
            tc.tile_pool(name="consts", bufs=1) as consts,
            tc.tile_pool(name="rsb", bufs=3) as rsb,
            tc.tile_pool(name="big", bufs=2) as big,
            tc.tile_pool(name="tree", bufs=2) as tree,
            tc.tile_pool(name="accs", bufs=2) as accs,
            tc.tile_pool(name="rp", bufs=4, space="PSUM") as rp,
            tc.tile_pool(name="sp", bufs=2, space="PSUM") as sp,
            tc.tile_pool(name="miscp", bufs=1, space="PSUM") as miscp,
            tc.tile_pool(name="extp", bufs=1, space="PSUM") as extp,
        ):
            # ---- weights / inputs into SBUF ----
            # lhs operands first (gate the matmuls), then slabs in use order
            # spread across the sync/scalar/gpsimd/vector DMA queues.
            ct_sb = consts.tile([CDIM, B], bf16)
            nc.sync.dma_start(out=ct_sb, in_=d_ct[:, :])
            cb_sb = consts.tile([BCH, 2, CDIM], bf16)
            for bc in range(2):
                nc.scalar.dma_start(
                    out=cb_sb[:, bc, :], in_=d_cb[bc * BCH : (bc + 1) * BCH, :]
                )
            xvt_sb = consts.tile([CDIM, 4, B], bf16)
            for kc in range(4):
                nc.gpsimd.dma_start(
                    out=xvt_sb[:, kc, :], in_=d_xvT[kc * 128 : (kc + 1) * 128, :]
                )
            ones_sb = consts.tile([1, BCH], bf16)
            nc.vector.memset(ones_sb, 1.0)

            wsl_sb = []
            qs = [nc.sync, nc.scalar, nc.gpsimd]
            si = 0
            for j in range(8):
                tiles_j = []
                for t, ext in enumerate(EXT_OF_J[j]):
                    tkn = consts.tile(
                        [ext, 8 * 128], bf16, name=f"wsl{j}_{t}", tag=f"wsl{j}_{t}"
                    )
                    qs[si % 3].dma_start(
                        out=tkn, in_=d_wsl[SLAB_BASE[si] : SLAB_BASE[si] + ext, :]
                    )
                    si += 1
                    tiles_j.append(tkn)
                wsl_sb.append(tiles_j)
            csl_sb = consts.tile([CDIM, NLOC * 128], bf16)
            for h in range(4):
                qs[h % 3].dma_start(
                    out=csl_sb[:, h * 2048 : (h + 1) * 2048],
                    in_=d_csl[:, h * 2048 : (h + 1) * 2048],
                )
            wdd_sb = consts.tile([CDIM, NLOC * FIN], bf16)
            nc.sync.dma_start(out=wdd_sb, in_=d_wdd[:, :])
            bdd_sb = consts.tile([1, NLOC * FIN], bf16)
            nc.sync.dma_start(out=bdd_sb, in_=d_bdd[:, :])
            ew_sb = consts.tile([CDIM, 3 * NLOC], bf16)
            nc.scalar.dma_start(out=ew_sb, in_=d_ew[:, :])
            eb_sb = consts.tile([1, 3 * NLOC], bf16)
            nc.scalar.dma_start(out=eb_sb, in_=d_eb[:, :])
            bdm_sb = consts.tile([CDIM, 4, NLOC], bf16)
            for kc in range(4):
                nc.gpsimd.dma_start(
                    out=bdm_sb[:, kc, :], in_=d_bdm[kc * 128 : (kc + 1) * 128, :]
                )
            xvd_sb, xle_sb = [], []
            for bc in range(2):
                b0 = bc * BCH
                tv = consts.tile([BCH, NLOC * FIN], bf16, name=f"xvd{bc}", tag=f"xvd{bc}")
                nc.sync.dma_start(out=tv, in_=d_xvd[b0 : b0 + BCH, :])
                xvd_sb.append(tv)
                tl = consts.tile([BCH, NLOC * FIN], bf16, name=f"xle{bc}", tag=f"xle{bc}")
                nc.scalar.dma_start(out=tl, in_=d_xle[b0 : b0 + BCH, :])
                xle_sb.append(tl)

            st = {}
            # ---- phase A: matmuls; S copies/squares; V mult+trees ----
            for bc in range(2):
                b0 = bc * BCH
                ctl = ct_sb[:, b0 : b0 + BCH]
                cbl = cb_sb[:, bc, :]

                # extras: wamp | bias | 2cg | dotbd
                pex = extp.tile([BCH, 4 * NLOC], f32, name="pex", tag="pex")
                nc.tensor.matmul(
                    pex[:, : 3 * NLOC], ctl, ew_sb, start=True, stop=False
                )
                nc.tensor.matmul(
                    pex[:, : 3 * NLOC], ones_sb, eb_sb, start=False, stop=True
                )
                for kc in range(4):
                    nc.tensor.matmul(
                        pex[:, 3 * NLOC :],
                        xvt_sb[:, kc, b0 : b0 + BCH],
                        bdm_sb[:, kc, :],
                        start=(kc == 0),
                        stop=(kc == 3),
                    )
                # diag block
                pdg = miscp.tile([BCH, NLOC * FIN], f32, name="pdg", tag="pdg")
                nc.tensor.matmul(pdg, ctl, wdd_sb, start=True, stop=False)
                nc.tensor.matmul(pdg, ones_sb, bdd_sb, start=False, stop=True)

                P = big.tile([BCH, NLOC * 128], bf16, name="P", tag="P")
                Q = big.tile([BCH, NLOC * 128], bf16, name="Q", tag="Q")
                for j in range(8):
                    nch = len(EXT_OF_J[j])
                    rpj = [None, None]
                    for h in range(2):
                        rpj[h] = rp.tile([BCH, 512], f32, name="rpj", tag="rpj")
                    # R matmuls: accumulate i-chunks with true K extent
                    for t, ext in enumerate(EXT_OF_J[j]):
                        for h in range(2):
                            nc.tensor.matmul(
                                rpj[h],
                                xvt_sb[:ext, t, b0 : b0 + BCH],
                                wsl_sb[j][t][:, h * 512 : (h + 1) * 512],
                                start=(t == 0),
                                stop=(t == nch - 1),
                            )
                    for h in range(2):
                        # chol matmul (single pass, K=128)
                        spj = sp.tile([BCH, 512], f32, name="spj", tag="spj")
                        c0 = j * 1024 + h * 512
                        nc.tensor.matmul(
                            spj, ctl, csl_sb[:, c0 : c0 + 512], start=True, stop=True
                        )
                        # S: psum -> sbuf bf16
                        rsj = rsb.tile([BCH, 512], bf16, name="rsj", tag="rsj")
                        nc.scalar.activation(out=rsj, in_=rpj[h], func=AF.Copy)
                        nc.scalar.activation(
                            out=Q[:, c0 : c0 + 512], in_=spj, func=AF.Square
                        )
                        # V: multiply by c (broadcast over the 4 rows of the half)
                        nc.vector.tensor_mul(
                            P[:, c0 : c0 + 512].rearrange("p (f k) -> p f k", k=CDIM),
                            rsj.rearrange("p (f k) -> p f k", k=CDIM),
                            cbl.unsqueeze(1).broadcast_to([BCH, 4, CDIM]),
                        )

                # add-trees: 128 -> 1 per row; last two levels in f32
                DOTL = accs.tile([BCH, NLOC], f32, name="DOTL", tag="DOTL")
                SQL = accs.tile([BCH, NLOC], f32, name="SQL", tag="SQL")
                for src, dst in ((P, DOTL), (Q, SQL)):
                    cur = src
                    w = 128
                    while w > 4:
                        w //= 2
                        nxt = tree.tile(
                            [BCH, NLOC * w], bf16, name=f"tr{w}", tag=f"tr{w}"
                        )
                        nc.vector.tensor_add(
                            nxt.rearrange("p (r k) -> p r k", k=w),
                            cur.rearrange("p (r k) -> p r k", k=2 * w)[:, :, :w],
                            cur.rearrange("p (r k) -> p r k", k=2 * w)[:, :, w:],
                        )
                        cur = nxt
                    t2 = tree.tile([BCH, NLOC * 2], f32, name="tr2", tag="tr2")
                    nc.vector.tensor_add(
                        t2.rearrange("p (r k) -> p r k", k=2),
                        cur.rearrange("p (r k) -> p r k", k=4)[:, :, :2],
                        cur.rearrange("p (r k) -> p r k", k=4)[:, :, 2:],
                    )
                    nc.vector.tensor_add(
                        dst,
                        t2.rearrange("p (r k) -> p r k", k=2)[:, :, 0],
                        t2.rearrange("p (r k) -> p r k", k=2)[:, :, 1],
                    )
                st[bc] = dict(pex=pex, pdg=pdg, DOTL=DOTL, SQL=SQL)

            # ---- phase B: diag elementwise ----
            for bc in range(2):
                s_ = st[bc]
                pdg = s_["pdg"]
                expd = tree.tile([BCH, NLOC * FIN], bf16, name="expd", tag="expd")
                nc.scalar.activation(out=expd, in_=pdg, func=AF.Exp)
                sq2 = tree.tile([BCH, NLOC * FIN], bf16, name="sq2", tag="sq2")
                nc.scalar.activation(out=sq2, in_=pdg, func=AF.Exp, scale=2.0)
                SQD = accs.tile([BCH, NLOC], f32, name="SQD", tag="SQD")
                nc.vector.tensor_reduce(
                    out=SQD,
                    in_=sq2.rearrange("p (r f) -> p r f", f=FIN),
                    axis=mybir.AxisListType.X,
                    op=ALU.add,
                )
                prd = tree.tile([BCH, NLOC * FIN], bf16, name="prd", tag="prd")
                nc.vector.tensor_mul(prd, expd, xvd_sb[bc])
                DOTD = accs.tile([BCH, NLOC], f32, name="DOTD", tag="DOTD")
                nc.vector.tensor_reduce(
                    out=DOTD,
                    in_=prd.rearrange("p (r f) -> p r f", f=FIN),
                    axis=mybir.AxisListType.X,
                    op=ALU.add,
                )
                prl = tree.tile([BCH, NLOC * FIN], bf16, name="prl", tag="prl")
                nc.vector.tensor_mul(prl, expd, xle_sb[bc])
                LDS = accs.tile([BCH, NLOC], f32, name="LDS", tag="LDS")
                nc.vector.tensor_reduce(
                    out=LDS,
                    in_=prl.rearrange("p (r f) -> p r f", f=FIN),
                    axis=mybir.AxisListType.X,
                    op=ALU.add,
                )
                s_.update(SQD=SQD, DOTD=DOTD, LDS=LDS)

            # ---- phase C: assembly ----
            for bc in range(2):
                s_ = st[bc]
                sqn = accs.tile([BCH, NLOC], f32, name="sqn", tag="sqn")
                nc.vector.tensor_add(sqn, s_["SQL"], s_["SQD"])
                nc.vector.tensor_add(sqn, sqn, s_["pex"][:, 2 * NLOC : 3 * NLOC])
                dot = accs.tile([BCH, NLOC], f32, name="dot", tag="dot")
                nc.vector.tensor_add(dot, s_["DOTL"], s_["DOTD"])
                nc.vector.tensor_add(dot, dot, s_["pex"][:, 3 * NLOC :])
                s_.update(sqn=sqn, dot=dot)
            for bc in range(2):
                s_ = st[bc]
                l1 = accs.tile([BCH, NLOC], f32, name="l1", tag="l1")
                nc.scalar.activation(out=l1, in_=s_["sqn"], func=AF.Ln)
                l2 = accs.tile([BCH, NLOC], f32, name="l2", tag="l2")
                nc.scalar.activation(out=l2, in_=s_["LDS"], func=AF.Ln)
                s_.update(l1=l1, l2=l2)
            for bc in range(2):
                s_ = st[bc]
                m1 = accs.tile([BCH, NLOC], f32, name="m1", tag="m1")
                nc.scalar.mul(m1, s_["l1"], -0.5)
                u = accs.tile([BCH, NLOC], f32, name="u", tag="u")
                nc.vector.tensor_add(u, s_["pex"][:, :NLOC], m1)
                s_.update(u=u)
            for bc in range(2):
                s_ = st[bc]
                sc = accs.tile([BCH, NLOC], f32, name="sc", tag="sc")
                nc.scalar.activation(out=sc, in_=s_["u"], func=AF.Exp)
                s_.update(sc=sc)
            for bc in range(2):
                b0 = bc * BCH
                s_ = st[bc]
                yv = accs.tile([BCH, NLOC], f32, name="yv", tag="yv")
                nc.vector.tensor_mul(yv, s_["dot"], s_["sc"])
                yb = accs.tile([BCH, NLOC], f32, name="yb", tag="yb")
                nc.vector.tensor_add(yb, yv, s_["pex"][:, NLOC : 2 * NLOC])
                ld = accs.tile([BCH, NLOC], f32, name="ld", tag="ld")
                nc.vector.tensor_add(ld, s_["u"], s_["l2"])
                ob = accs.tile([BCH, NLOC, 2], f32, name="ob", tag="ob")
                nc.gpsimd.tensor_copy(out=ob[:, :, 0], in_=yb)
                nc.gpsimd.tensor_copy(out=ob[:, :, 1], in_=ld)
                nc.sync.dma_start(out=d_out[b0 : b0 + BCH, :, :], in_=ob)

    nc.compile()
    return nc


def _host_prep(x, c, Wd, bd, Wa, ba, Wb, bb):
    """Build the 8 per-core input maps."""
    import ml_dtypes

    bf = ml_dtypes.bfloat16
    x = np.ascontiguousarray(x, dtype=np.float32)
    c = np.ascontiguousarray(c, dtype=np.float32)
    Wd5 = np.ascontiguousarray(Wd, dtype=np.float32).reshape(CDIM, NCH, FOUT, NCH, FIN)
    bd4 = np.ascontiguousarray(bd, dtype=np.float32).reshape(NCH, FOUT, NCH, FIN)
    Wa = np.ascontiguousarray(Wa, dtype=np.float32)
    Wb = np.ascontiguousarray(Wb, dtype=np.float32)
    ba = np.ascontiguousarray(ba, dtype=np.float32)
    bb = np.ascontiguousarray(bb, dtype=np.float32)

    cT = np.ascontiguousarray(c.T)
    xv = np.ascontiguousarray(x[:, :, 0])
    xl = np.ascontiguousarray(x[:, :, 1])
    xvT = np.ascontiguousarray(xv.T)

    in_maps = []
    for k in range(NCORES):
        chs = _channels(k)
        wsl = np.zeros((TOT_SLAB, 8 * 128), dtype=np.float32)
        csl = np.zeros((CDIM, NLOC * 128), dtype=np.float32)
        wdd = np.empty((CDIM, NLOC * FIN), dtype=np.float32)
        bdd = np.empty((1, NLOC * FIN), dtype=np.float32)
        ew = np.zeros((CDIM, 3 * NLOC), dtype=np.float32)
        eb = np.zeros((1, 3 * NLOC), dtype=np.float32)
        bdm = np.zeros((I, NLOC), dtype=np.float32)
        xvd = np.empty((B, NLOC * FIN), dtype=np.float32)
        xled = np.empty((B, NLOC * FIN), dtype=np.float32)
        si = 0
        for j, ch in enumerate(chs):
            w = ch * FIN
            Gall = Wd5[:, ch, :, :, :].reshape(CDIM, FOUT, I)  # [128, 8, 512]
            for t, ext in enumerate(EXT_OF_J[j]):
                i0 = 128 * t
                n = max(0, min(w - i0, ext))
                if n > 0:
                    # slab[ii, fo*128+kk] = Wd[kk, (ch,fo), i0+ii]
                    seg = Gall[:, :, i0 : i0 + n]  # [128, 8, n]
                    wsl[SLAB_BASE[si] : SLAB_BASE[si] + n, :] = (
                        seg.transpose(2, 1, 0).reshape(n, 8 * 128)
                    )
                si += 1
            for fo in range(FOUT):
                r = j * FOUT + fo
                G = Gall[:, fo, :w]  # [128, w]
                if w > 0:
                    if w <= 128:
                        csl[:, r * 128 : r * 128 + w] = G
                    else:
                        M = (G.astype(np.float64) @ G.astype(np.float64).T)
                        M += np.eye(CDIM) * (1e-12 * np.trace(M) / CDIM)
                        csl[:, r * 128 : (r + 1) * 128] = np.linalg.cholesky(M)
                wdd[:, r * FIN : (r + 1) * FIN] = Wd5[:, ch, fo, ch, :]
                bdd[0, r * FIN : (r + 1) * FIN] = bd4[ch, fo, ch, :]
                bd_low = bd4[ch, fo, :ch, :].reshape(-1)
                bdm[:w, r] = bd_low
                if w > 0:
                    ew[:, 2 * NLOC + r] = 2.0 * (G @ bd_low)
                    eb[0, 2 * NLOC + r] = np.dot(bd_low, bd_low)
            rows = slice(ch * FOUT, (ch + 1) * FOUT)
            ew[:, j * FOUT : (j + 1) * FOUT] = Wa[:, rows]
            ew[:, NLOC + j * FOUT : NLOC + (j + 1) * FOUT] = Wb[:, rows]
            eb[0, j * FOUT : (j + 1) * FOUT] = ba[rows]
            eb[0, NLOC + j * FOUT : NLOC + (j + 1) * FOUT] = bb[rows]
        for r in range(NLOC):
            j = r // FOUT
            ch = chs[j]
            xvd[:, r * FIN : (r + 1) * FIN] = xv[:, ch * FIN : (ch + 1) * FIN]
            xled[:, r * FIN : (r + 1) * FIN] = xl[:, ch * FIN : (ch + 1) * FIN]
        np.exp(xled, out=xled)

        in_maps.append(
            {
                "ct": cT.astype(bf),
                "cb": c.astype(bf),
                "xvt": xvT.astype(bf),
                "wsl": wsl.astype(bf),
                "csl": csl.astype(bf),
                "wdd": wdd.astype(bf),
                "bdd": bdd.astype(bf),
                "ew": ew.astype(bf),
                "eb": eb.astype(bf),
                "bdm": bdm.astype(bf),
                "xvd": xvd.astype(bf),
                "xle": xled.astype(bf),
            }
        )
    return in_maps


def kernel(x, c, Wd, bd, Wa, ba, Wb, bb, _trace=False, _tmpdir=None):
    global _NC
    from concourse.bass_utils import run_bass_kernel_spmd

    if _NC is None:
        _NC = _build_nc()
    in_maps = _host_prep(x, c, Wd, bd, Wa, ba, Wb, bb)
    res = run_bass_kernel_spmd(
        _NC, in_maps, core_ids=list(range(NCORES)), trace=_trace, tmpdir=_tmpdir
    )

    out = np.empty((B, O, 2), dtype=np.float32)
    for k in range(NCORES):
        ok = res.results[k]["out"]
        for j, ch in enumerate(_channels(k)):
            out[:, ch * FOUT : (ch + 1) * FOUT, :] = ok[:, j * FOUT : (j + 1) * FOUT, :]
    if _trace:
        return out, res
    return out


# revision 9
# speedup vs baseline: 1.1070x; 1.0084x over previous
"""Trainium2 Bass kernel for nn_CLinear_6768868459230.

Context-conditioned block-autoregressive linear layer (MAF-style):
  wdir = c @ Wd + bd                      [B, O, I]
  w    = exp(wdir)*mask_diag + wdir*mask_lower
  sqn  = sum(w^2, axis=I)
  y    = (w / sqrt(sqn) * exp(wamp)) @ xv + bias
  logdet = logsumexp over diag block of (wdir - 0.5 log sqn + wamp + xl)

Sharding: tensor-parallel over the O=512 output rows; core k owns channels
{k, 15-k, 16+k, 31-k, ...} so the triangular work is the same on every core.

Algorithm (v3): never materialize t = c @ Wd_lower (whose [B, ~17k]
elementwise reductions bound the baseline).  Instead per output row o:
  dot_lower = c^T (W_o x)   -> R[b,(o,k)] = sum_i W[k,o,i] x[b,i] on
     TensorE (i-chunked, true-K, no padding cost), then one multiply by c
     and a bf16 add-tree: 128 terms/row instead of up to 504.
  sqn_lower = |C_o^T c|^2 + 2(G_o bd)^T c + |bd|^2, C_o = G_o (w<=128) or
     chol(G_o G_o^T): ScalarE squares the 128-wide s = C^T c straight out
     of PSUM (bf16 out), GpSimd segment-reduces.
Engine split per 128-sample chunk: TensorE ~30k cols; PSUM evacuation
split S (squares + 4 slot copies) / V (4 direct-PSUM mults); V runs the
bf16 dot-tree; GpSimd the square-reduce, diag products and output
interleave.  All operands bf16; DMAs consolidated into ~14 transfers.
"""

import numpy as np

NCH, FIN, FOUT, CDIM, B = 64, 8, 8, 128, 256
I = NCH * FIN
O = NCH * FOUT
NCORES = 8
NLOC = 64  # output rows per core
BCH = 128  # batch chunk (SBUF partitions)

# per-slot i-chunk extents (max over cores of w=8*ch for that slot)
EXT_OF_J = [
    [56],
    [120],
    [128, 56],
    [128, 120],
    [128, 128, 56],
    [128, 128, 120],
    [128, 128, 128, 56],
    [128, 128, 128, 120],
]
# slab packing: full 128-row slabs in one [128, 13*1024] tensor; the
# 56-row and 120-row tails in [56, 4*1024] and [120, 4*1024] tensors.
SLAB_PACK = {}
_nf = _n56 = _n120 = 0
for _j in range(8):
    for _t, _e in enumerate(EXT_OF_J[_j]):
        if _e == 128:
            SLAB_PACK[(_j, _t)] = ("f", _nf * 1024)
            _nf += 1
        elif _e == 56:
            SLAB_PACK[(_j, _t)] = ("a", _n56 * 1024)
            _n56 += 1
        else:
            SLAB_PACK[(_j, _t)] = ("b", _n120 * 1024)
            _n120 += 1

# col offsets inside the consolidated [128, NCOL] blob
C_CT = 0          # [128, 256]   c transposed
C_XVT = 256       # [128, 4*256] xv transposed, 4 i-chunks
C_CB8 = 1280      # [128, 2*1024] c tiled 8x per batch chunk
C_CSL = 3328      # [128, 64*128] chol/G factors
C_WDD = 11520     # [128, 512]   diag weights
C_BDM = 12032     # [128, 4*64]  bd-lower matvec weights, 4 i-chunks
C_XVD = 12288     # [128, 2*512] diag-gathered xv per batch chunk
C_XLE = 13312     # [128, 2*512] diag-gathered exp(xl)
C_EW = 14336      # [128, 192]   wamp | bias | 2cg weights
NCOL = 14528


def _channels(k):
    return [k, 15 - k, 16 + k, 31 - k, 32 + k, 47 - k, 48 + k, 63 - k]


_NC = None


def _build_nc():
    import concourse.bacc as bacc
    import concourse.tile as tile
    from concourse import mybir

    f32 = mybir.dt.float32
    bf16 = mybir.dt.bfloat16
    AF = mybir.ActivationFunctionType
    ALU = mybir.AluOpType

    nc = bacc.Bacc(None, target_bir_lowering=False)

    d_blob = nc.dram_tensor("blob", [CDIM, NCOL], bf16, kind="ExternalInput")
    d_row1 = nc.dram_tensor("row1", [1, 704], bf16, kind="ExternalInput")
    d_wf = nc.dram_tensor("wf", [128, 13 * 1024], bf16, kind="ExternalInput")
    d_wa = nc.dram_tensor("wa", [56, 4 * 1024], bf16, kind="ExternalInput")
    d_wb = nc.dram_tensor("wb", [120, 4 * 1024], bf16, kind="ExternalInput")
    d_out = nc.dram_tensor("out", [B, NLOC, 2], f32, kind="ExternalOutput")

    with tile.TileContext(nc) as tc:
        with (
            tc.tile_pool(name="consts", bufs=1) as consts,
            tc.tile_pool(name="rsb", bufs=3) as rsb,
            tc.tile_pool(name="big", bufs=2) as big,
            tc.tile_pool(name="tree", bufs=1) as tree,
            tc.tile_pool(name="accs", bufs=2) as accs,
            tc.tile_pool(name="rp", bufs=2, space="PSUM") as rp,
            tc.tile_pool(name="sp", bufs=1, space="PSUM") as sp,
            tc.tile_pool(name="miscp", bufs=1, space="PSUM") as miscp,
            tc.tile_pool(name="extp", bufs=1, space="PSUM") as extp,
        ):
            blob = consts.tile([CDIM, NCOL], bf16)
            # first-used columns first, spread over the three DGE queues
            nc.sync.dma_start(out=blob[:, :C_CSL], in_=d_blob[:, :C_CSL])
            nc.scalar.dma_start(
                out=blob[:, C_CSL : C_CSL + 4096], in_=d_blob[:, C_CSL : C_CSL + 4096]
            )
            nc.gpsimd.dma_start(
                out=blob[:, C_CSL + 4096 : C_WDD],
                in_=d_blob[:, C_CSL + 4096 : C_WDD],
            )
            nc.scalar.dma_start(out=blob[:, C_WDD:], in_=d_blob[:, C_WDD:])
            row1 = consts.tile([1, 704], bf16)
            nc.gpsimd.dma_start(out=row1, in_=d_row1[:, :])
            ones_sb = consts.tile([1, BCH], bf16)
            nc.vector.memset(ones_sb, 1.0)

            wf = consts.tile([128, 13 * 1024], bf16)
            nc.sync.dma_start(out=wf[:, : 5 * 1024], in_=d_wf[:, : 5 * 1024])
            nc.scalar.dma_start(
                out=wf[:, 5 * 1024 : 9 * 1024], in_=d_wf[:, 5 * 1024 : 9 * 1024]
            )
            nc.gpsimd.dma_start(out=wf[:, 9 * 1024 :], in_=d_wf[:, 9 * 1024 :])
            wa = consts.tile([56, 4 * 1024], bf16)
            nc.sync.dma_start(out=wa, in_=d_wa[:, :])
            wb = consts.tile([120, 4 * 1024], bf16)
            nc.scalar.dma_start(out=wb, in_=d_wb[:, :])
            wtile = {"f": wf, "a": wa, "b": wb}

            st = {}
            # ---- phase A ----
            for bc in range(2):
                b0 = bc * BCH
                ctl = blob[:, C_CT + b0 : C_CT + b0 + BCH]
                cb8 = blob[:, C_CB8 + bc * 1024 : C_CB8 + (bc + 1) * 1024]

                # extras: wamp | bias | 2cg | dotbd
                pex = extp.tile([BCH, 4 * NLOC], f32, name="pex", tag="pex")
                nc.tensor.matmul(
                    pex[:, : 3 * NLOC],
                    ctl,
                    blob[:, C_EW : C_EW + 192],
                    start=True,
                    stop=False,
                )
                nc.tensor.matmul(
                    pex[:, : 3 * NLOC],
                    ones_sb,
                    row1[:, 512:704],
                    start=False,
                    stop=True,
                )
                # diag block
                pdg = miscp.tile([BCH, NLOC * FIN], f32, name="pdg", tag="pdg")
                nc.tensor.matmul(
                    pdg, ctl, blob[:, C_WDD : C_WDD + 512], start=True, stop=False
                )
                nc.tensor.matmul(pdg, ones_sb, row1[:, :512], start=False, stop=True)

                P = big.tile([BCH, NLOC * 128], bf16, name="P", tag="P")
                Q = big.tile([BCH, NLOC * 128], bf16, name="Q", tag="Q")
                for j in range(8):
                    c0 = j * 1024
                    # chol matmul first (S consumes while R matmuls run)
                    spj = sp.tile([BCH, 1024], f32, name="spj", tag="spj")
                    for h in range(2):
                        nc.tensor.matmul(
                            spj[:, h * 512 : (h + 1) * 512],
                            ctl,
                            blob[
                                :, C_CSL + c0 + h * 512 : C_CSL + c0 + (h + 1) * 512
                            ],
                            start=True,
                            stop=True,
                        )
                    rpj = rp.tile([BCH, 1024], f32, name="rpj", tag="rpj")
                    nch = len(EXT_OF_J[j])
                    for t, ext in enumerate(EXT_OF_J[j]):
                        kind, off = SLAB_PACK[(j, t)]
                        wt = wtile[kind]
                        xcol = C_XVT + t * 256 + b0
                        for h in range(2):
                            nc.tensor.matmul(
                                rpj[:, h * 512 : (h + 1) * 512],
                                blob[:ext, xcol : xcol + BCH],
                                wt[:ext, off + h * 512 : off + (h + 1) * 512],
                                start=(t == 0),
                                stop=(t == nch - 1),
                            )
                    # S: square the chol output straight out of PSUM
                    nc.scalar.activation(
                        out=Q[:, c0 : c0 + 1024], in_=spj, func=AF.Square
                    )
                    if j < 4:
                        # S copies R to SBUF; V multiplies at 2x
                        rsj = rsb.tile([BCH, 1024], bf16, name="rsj", tag="rsj")
                        nc.scalar.activation(out=rsj, in_=rpj, func=AF.Copy)
                        nc.vector.tensor_mul(P[:, c0 : c0 + 1024], rsj, cb8)
                    else:
                        # V multiplies straight out of PSUM (1x)
                        nc.vector.tensor_mul(P[:, c0 : c0 + 1024], rpj, cb8)

                # dotbd into pex[:, 192:256]
                for kc in range(4):
                    xcol = C_XVT + kc * 256 + b0
                    nc.tensor.matmul(
                        pex[:, 3 * NLOC :],
                        blob[:, xcol : xcol + BCH],
                        blob[:, C_BDM + kc * 64 : C_BDM + (kc + 1) * 64],
                        start=(kc == 0),
                        stop=(kc == 3),
                    )

                # dot add-tree on V (128 -> 1 per row, last two levels f32)
                DOTL = accs.tile([BCH, NLOC], f32, name="DOTL", tag="DOTL")
                cur = P
                w = 128
                while w > 4:
                    w //= 2
                    nxt = tree.tile([BCH, NLOC * w], bf16, name=f"tr{w}", tag=f"tr{w}")
                    nc.vector.tensor_add(
                        nxt.rearrange("p (r k) -> p r k", k=w),
                        cur.rearrange("p (r k) -> p r k", k=2 * w)[:, :, :w],
                        cur.rearrange("p (r k) -> p r k", k=2 * w)[:, :, w:],
                    )
                    cur = nxt
                t2 = tree.tile([BCH, NLOC * 2], f32, name="tr2", tag="tr2")
                nc.vector.tensor_add(
                    t2.rearrange("p (r k) -> p r k", k=2),
                    cur.rearrange("p (r k) -> p r k", k=4)[:, :, :2],
                    cur.rearrange("p (r k) -> p r k", k=4)[:, :, 2:],
                )
                nc.vector.tensor_add(
                    DOTL,
                    t2.rearrange("p (r k) -> p r k", k=2)[:, :, 0],
                    t2.rearrange("p (r k) -> p r k", k=2)[:, :, 1],
                )
                # square add-tree on GpSimd
                SQL = accs.tile([BCH, NLOC], f32, name="SQL", tag="SQL")
                cur = Q
                w = 128
                while w > 4:
                    w //= 2
                    nxt = tree.tile([BCH, NLOC * w], bf16, name=f"sq{w}", tag=f"sq{w}")
                    nc.gpsimd.tensor_add(
                        nxt.rearrange("p (r k) -> p r k", k=w),
                        cur.rearrange("p (r k) -> p r k", k=2 * w)[:, :, :w],
                        cur.rearrange("p (r k) -> p r k", k=2 * w)[:, :, w:],
                    )
                    cur = nxt
                q2 = tree.tile([BCH, NLOC * 2], f32, name="sq2l", tag="sq2l")
                nc.gpsimd.tensor_add(
                    q2.rearrange("p (r k) -> p r k", k=2),
                    cur.rearrange("p (r k) -> p r k", k=4)[:, :, :2],
                    cur.rearrange("p (r k) -> p r k", k=4)[:, :, 2:],
                )
                nc.gpsimd.tensor_add(
                    SQL,
                    q2.rearrange("p (r k) -> p r k", k=2)[:, :, 0],
                    q2.rearrange("p (r k) -> p r k", k=2)[:, :, 1],
                )
                st[bc] = dict(pex=pex, pdg=pdg, DOTL=DOTL, SQL=SQL)

            # ---- phase B: diag elementwise ----
            for bc in range(2):
                s_ = st[bc]
                pdg = s_["pdg"]
                xvd = blob[:, C_XVD + bc * 512 : C_XVD + (bc + 1) * 512]
                xle = blob[:, C_XLE + bc * 512 : C_XLE + (bc + 1) * 512]
                expd = tree.tile([BCH, NLOC * FIN], bf16, name="expd", tag="expd")
                nc.scalar.activation(out=expd, in_=pdg, func=AF.Exp)
                sq2 = tree.tile([BCH, NLOC * FIN], bf16, name="sq2", tag="sq2")
                nc.scalar.activation(out=sq2, in_=pdg, func=AF.Exp, scale=2.0)
                SQD = accs.tile([BCH, NLOC], f32, name="SQD", tag="SQD")
                nc.vector.tensor_reduce(
                    out=SQD,
                    in_=sq2.rearrange("p (r f) -> p r f", f=FIN),
                    axis=mybir.AxisListType.X,
                    op=ALU.add,
                )
                prd = tree.tile([BCH, NLOC * FIN], bf16, name="prd", tag="prd")
                nc.gpsimd.tensor_mul(prd, expd, xvd)
                DOTD = accs.tile([BCH, NLOC], f32, name="DOTD", tag="DOTD")
                nc.vector.tensor_reduce(
                    out=DOTD,
                    in_=prd.rearrange("p (r f) -> p r f", f=FIN),
                    axis=mybir.AxisListType.X,
                    op=ALU.add,
                )
                prl = tree.tile([BCH, NLOC * FIN], bf16, name="prl", tag="prl")
                nc.gpsimd.tensor_mul(prl, expd, xle)
                LDS = accs.tile([BCH, NLOC], f32, name="LDS", tag="LDS")
                nc.vector.tensor_reduce(
                    out=LDS,
                    in_=prl.rearrange("p (r f) -> p r f", f=FIN),
                    axis=mybir.AxisListType.X,
                    op=ALU.add,
                )
                s_.update(SQD=SQD, DOTD=DOTD, LDS=LDS)

            # ---- phase C: assembly ----
            for bc in range(2):
                s_ = st[bc]
                sqn = accs.tile([BCH, NLOC], f32, name="sqn", tag="sqn")
                nc.vector.tensor_add(sqn, s_["SQL"], s_["SQD"])
                nc.vector.tensor_add(sqn, sqn, s_["pex"][:, 2 * NLOC : 3 * NLOC])
                dot = accs.tile([BCH, NLOC], f32, name="dot", tag="dot")
                nc.vector.tensor_add(dot, s_["DOTL"], s_["DOTD"])
                nc.vector.tensor_add(dot, dot, s_["pex"][:, 3 * NLOC :])
                s_.update(sqn=sqn, dot=dot)
            for bc in range(2):
                s_ = st[bc]
                l1 = accs.tile([BCH, NLOC], f32, name="l1", tag="l1")
                nc.scalar.activation(out=l1, in_=s_["sqn"], func=AF.Ln)
                l2 = accs.tile([BCH, NLOC], f32, name="l2", tag="l2")
                nc.scalar.activation(out=l2, in_=s_["LDS"], func=AF.Ln)
                s_.update(l1=l1, l2=l2)
            for bc in range(2):
                s_ = st[bc]
                m1 = accs.tile([BCH, NLOC], f32, name="m1", tag="m1")
                nc.scalar.mul(m1, s_["l1"], -0.5)
                u = accs.tile([BCH, NLOC], f32, name="u", tag="u")
                nc.vector.tensor_add(u, s_["pex"][:, :NLOC], m1)
                s_.update(u=u)
            for bc in range(2):
                s_ = st[bc]
                sc = accs.tile([BCH, NLOC], f32, name="sc", tag="sc")
                nc.scalar.activation(out=sc, in_=s_["u"], func=AF.Exp)
                s_.update(sc=sc)
            for bc in range(2):
                b0 = bc * BCH
                s_ = st[bc]
                yv = accs.tile([BCH, NLOC], f32, name="yv", tag="yv")
                nc.vector.tensor_mul(yv, s_["dot"], s_["sc"])
                yb = accs.tile([BCH, NLOC], f32, name="yb", tag="yb")
                nc.vector.tensor_add(yb, yv, s_["pex"][:, NLOC : 2 * NLOC])
                ld = accs.tile([BCH, NLOC], f32, name="ld", tag="ld")
                nc.vector.tensor_add(ld, s_["u"], s_["l2"])
                ob = accs.tile([BCH, NLOC, 2], f32, name="ob", tag="ob")
                nc.gpsimd.tensor_copy(out=ob[:, :, 0], in_=yb)
                nc.gpsimd.tensor_copy(out=ob[:, :, 1], in_=ld)
                nc.sync.dma_start(out=d_out[b0 : b0 + BCH, :, :], in_=ob)

    nc.compile()
    return nc


def _host_prep(x, c, Wd, bd, Wa, ba, Wb, bb):
    """Build the 8 per-core input maps."""
    import ml_dtypes

    bf = ml_dtypes.bfloat16
    x = np.ascontiguousarray(x, dtype=np.float32)
    c = np.ascontiguousarray(c, dtype=np.float32)
    Wd5 = np.ascontiguousarray(Wd, dtype=np.float32).reshape(CDIM, NCH, FOUT, NCH, FIN)
    bd4 = np.ascontiguousarray(bd, dtype=np.float32).reshape(NCH, FOUT, NCH, FIN)
    Wa_ = np.ascontiguousarray(Wa, dtype=np.float32)
    Wb_ = np.ascontiguousarray(Wb, dtype=np.float32)
    ba = np.ascontiguousarray(ba, dtype=np.float32)
    bb = np.ascontiguousarray(bb, dtype=np.float32)

    cT = np.ascontiguousarray(c.T)
    xv = np.ascontiguousarray(x[:, :, 0])
    xl = np.ascontiguousarray(x[:, :, 1])
    xvT = np.ascontiguousarray(xv.T)

    in_maps = []
    for k in range(NCORES):
        chs = _channels(k)
        blob = np.zeros((CDIM, NCOL), dtype=np.float32)
        row1 = np.zeros((1, 704), dtype=np.float32)
        wfp = np.zeros((128, 13 * 1024), dtype=np.float32)
        wap = np.zeros((56, 4 * 1024), dtype=np.float32)
        wbp = np.zeros((120, 4 * 1024), dtype=np.float32)
        packs = {"f": wfp, "a": wap, "b": wbp}

        blob[:, C_CT : C_CT + 256] = cT
        for t in range(4):
            blob[:, C_XVT + t * 256 : C_XVT + (t + 1) * 256] = xvT[
                t * 128 : (t + 1) * 128, :
            ]
        for bc in range(2):
            blob[:, C_CB8 + bc * 1024 : C_CB8 + (bc + 1) * 1024] = np.tile(
                c[bc * BCH : (bc + 1) * BCH, :], (1, 8)
            )

        bdm = np.zeros((I, NLOC), dtype=np.float32)
        for j, ch in enumerate(chs):
            w = ch * FIN
            Gall = Wd5[:, ch, :, :, :].reshape(CDIM, FOUT, I)  # [128, 8, 512]
            for t, ext in enumerate(EXT_OF_J[j]):
                i0 = 128 * t
                n = max(0, min(w - i0, ext))
                if n > 0:
                    kind, off = SLAB_PACK[(j, t)]
                    seg = Gall[:, :, i0 : i0 + n]  # [128, 8, n]
                    packs[kind][:n, off : off + 1024] = seg.transpose(2, 1, 0).reshape(
                        n, 8 * 128
                    )
            for fo in range(FOUT):
                r = j * FOUT + fo
                G = Gall[:, fo, :w]  # [128, w]
                if w > 0:
                    if w <= 128:
                        blob[:, C_CSL + r * 128 : C_CSL + r * 128 + w] = G
                    else:
                        M = G.astype(np.float64) @ G.astype(np.float64).T
                        M += np.eye(CDIM) * (1e-12 * np.trace(M) / CDIM)
                        blob[:, C_CSL + r * 128 : C_CSL + (r + 1) * 128] = (
                            np.linalg.cholesky(M)
                        )
                blob[:, C_WDD + r * FIN : C_WDD + (r + 1) * FIN] = Wd5[:, ch, fo, ch, :]
                row1[0, r * FIN : (r + 1) * FIN] = bd4[ch, fo, ch, :]
                bd_low = bd4[ch, fo, :ch, :].reshape(-1)
                bdm[:w, r] = bd_low
                if w > 0:
                    blob[:, C_EW + 128 + r] = 2.0 * (G @ bd_low)
                    row1[0, 512 + 128 + r] = np.dot(bd_low, bd_low)
            rows = slice(ch * FOUT, (ch + 1) * FOUT)
            blob[:, C_EW + j * FOUT : C_EW + (j + 1) * FOUT] = Wa_[:, rows]
            blob[:, C_EW + 64 + j * FOUT : C_EW + 64 + (j + 1) * FOUT] = Wb_[:, rows]
            row1[0, 512 + j * FOUT : 512 + (j + 1) * FOUT] = ba[rows]
            row1[0, 512 + 64 + j * FOUT : 512 + 64 + (j + 1) * FOUT] = bb[rows]

        for kc in range(4):
            blob[:, C_BDM + kc * 64 : C_BDM + (kc + 1) * 64] = bdm[
                kc * 128 : (kc + 1) * 128, :
            ]

        xvd = np.empty((B, NLOC * FIN), dtype=np.float32)
        xled = np.empty((B, NLOC * FIN), dtype=np.float32)
        for r in range(NLOC):
            ch = chs[r // FOUT]
            xvd[:, r * FIN : (r + 1) * FIN] = xv[:, ch * FIN : (ch + 1) * FIN]
            xled[:, r * FIN : (r + 1) * FIN] = xl[:, ch * FIN : (ch + 1) * FIN]
        np.exp(xled, out=xled)
        for bc in range(2):
            blob[:, C_XVD + bc * 512 : C_XVD + (bc + 1) * 512] = xvd[
                bc * BCH : (bc + 1) * BCH, :
            ]
            blob[:, C_XLE + bc * 512 : C_XLE + (bc + 1) * 512] = xled[
                bc * BCH : (bc + 1) * BCH, :
            ]

        in_maps.append(
            {
                "blob": blob.astype(bf),
                "row1": row1.astype(bf),
                "wf": wfp.astype(bf),
                "wa": wap.astype(bf),
                "wb": wbp.astype(bf),
            }
        )
    return in_maps


def kernel(x, c, Wd, bd, Wa, ba, Wb, bb, _trace=False, _tmpdir=None):
    global _NC
    from concourse.bass_utils import run_bass_kernel_spmd

    if _NC is None:
        _NC = _build_nc()
    in_maps = _host_prep(x, c, Wd, bd, Wa, ba, Wb, bb)
    res = run_bass_kernel_spmd(
        _NC, in_maps, core_ids=list(range(NCORES)), trace=_trace, tmpdir=_tmpdir
    )

    out = np.empty((B, O, 2), dtype=np.float32)
    for k in range(NCORES):
        ok = res.results[k]["out"]
        for j, ch in enumerate(_channels(k)):
            out[:, ch * FOUT : (ch + 1) * FOUT, :] = ok[:, j * FOUT : (j + 1) * FOUT, :]
    if _trace:
        return out, res
    return out


# revision 10
# speedup vs baseline: 1.3554x; 1.2243x over previous
"""Trainium2 Bass kernel for nn_CLinear_6768868459230.

Context-conditioned block-autoregressive linear layer (MAF-style):
  wdir = c @ Wd + bd                      [B, O, I]
  w    = exp(wdir)*mask_diag + wdir*mask_lower
  sqn  = sum(w^2, axis=I)
  y    = (w / sqrt(sqn) * exp(wamp)) @ xv + bias
  logdet = logsumexp over diag block of (wdir - 0.5 log sqn + wamp + xl)

Sharding: tensor-parallel over the O=512 output rows; core k owns channels
{k, 15-k, 16+k, 31-k, ...} so the triangular work is the same on every core.

Algorithm (v3): never materialize t = c @ Wd_lower (whose [B, ~17k]
elementwise reductions bound the baseline).  Instead per output row o:
  dot_lower = c^T (W_o x)   -> R[b,(o,k)] = sum_i W[k,o,i] x[b,i] on
     TensorE (i-chunked, true-K, no padding cost), then one multiply by c
     and a bf16 add-tree: 128 terms/row instead of up to 504.
  sqn_lower = |C_o^T c|^2 + 2(G_o bd)^T c + |bd|^2, C_o = G_o (w<=128) or
     chol(G_o G_o^T): ScalarE squares the 128-wide s = C^T c straight out
     of PSUM (bf16 out), GpSimd segment-reduces.
Engine split per 128-sample chunk: TensorE ~30k cols; PSUM evacuation
split S (squares + 4 slot copies) / V (4 direct-PSUM mults); V runs the
bf16 dot-tree; GpSimd the square-reduce, diag products and output
interleave.  All operands bf16; DMAs consolidated into ~14 transfers.
"""

import numpy as np

NCH, FIN, FOUT, CDIM, B = 64, 8, 8, 128, 256
I = NCH * FIN
O = NCH * FOUT
NCORES = 8
NLOC = 64  # output rows per core
BCH = 128  # batch chunk (SBUF partitions)

# per-slot i-chunk extents (max over cores of w=8*ch for that slot)
EXT_OF_J = [
    [56],
    [120],
    [128, 56],
    [128, 120],
    [128, 128, 56],
    [128, 128, 120],
    [128, 128, 128, 56],
    [128, 128, 128, 120],
]
# slab packing: full 128-row slabs in one [128, 13*1024] tensor; the
# 56-row and 120-row tails in [56, 4*1024] and [120, 4*1024] tensors.
SLAB_PACK = {}
_nf = _n56 = _n120 = 0
for _j in range(8):
    for _t, _e in enumerate(EXT_OF_J[_j]):
        if _e == 128:
            SLAB_PACK[(_j, _t)] = ("f", _nf * 1024)
            _nf += 1
        elif _e == 56:
            SLAB_PACK[(_j, _t)] = ("a", _n56 * 1024)
            _n56 += 1
        else:
            SLAB_PACK[(_j, _t)] = ("b", _n120 * 1024)
            _n120 += 1

# col offsets inside the consolidated [128, NCOL] blob
C_CT = 0          # [128, 256]   c transposed
C_XVT = 256       # [128, 4*256] xv transposed, 4 i-chunks
C_CB8 = 1280      # [128, 2*1024] c tiled 8x per batch chunk
C_CSL = 3328      # [128, 64*128] chol/G factors
C_WDD = 11520     # [128, 512]   diag weights
C_BDM = 12032     # [128, 4*64]  bd-lower matvec weights, 4 i-chunks
C_XVD = 12288     # [128, 2*512] diag-gathered xv per batch chunk
C_XLE = 13312     # [128, 2*512] diag-gathered exp(xl)
C_EW = 14336      # [128, 192]   wamp | bias | 2cg weights
NCOL = 14528


def _channels(k):
    return [k, 15 - k, 16 + k, 31 - k, 32 + k, 47 - k, 48 + k, 63 - k]


_NC = None


def _build_nc():
    import concourse.bacc as bacc
    import concourse.tile as tile
    from concourse import mybir

    f32 = mybir.dt.float32
    bf16 = mybir.dt.bfloat16
    AF = mybir.ActivationFunctionType
    ALU = mybir.AluOpType

    nc = bacc.Bacc(None, target_bir_lowering=False)

    d_blob = nc.dram_tensor("blob", [CDIM, NCOL], bf16, kind="ExternalInput")
    d_row1 = nc.dram_tensor("row1", [1, 704], bf16, kind="ExternalInput")
    d_wf = nc.dram_tensor("wf", [128, 13 * 1024], bf16, kind="ExternalInput")
    d_wa = nc.dram_tensor("wa", [56, 4 * 1024], bf16, kind="ExternalInput")
    d_wb = nc.dram_tensor("wb", [120, 4 * 1024], bf16, kind="ExternalInput")
    d_out = nc.dram_tensor("out", [B, NLOC, 2], f32, kind="ExternalOutput")

    with tile.TileContext(nc) as tc:
        with (
            tc.tile_pool(name="consts", bufs=1) as consts,
            tc.tile_pool(name="rsb", bufs=3) as rsb,
            tc.tile_pool(name="big", bufs=2) as big,
            tc.tile_pool(name="tree", bufs=1) as tree,
            tc.tile_pool(name="accs", bufs=2) as accs,
            tc.tile_pool(name="rp", bufs=2, space="PSUM") as rp,
            tc.tile_pool(name="sp", bufs=1, space="PSUM") as sp,
            tc.tile_pool(name="miscp", bufs=1, space="PSUM") as miscp,
            tc.tile_pool(name="extp", bufs=1, space="PSUM") as extp,
        ):
            blob = consts.tile([CDIM, NCOL], bf16)
            # first-used columns first, spread over the three DGE queues
            nc.sync.dma_start(out=blob[:, :C_CSL], in_=d_blob[:, :C_CSL])
            nc.scalar.dma_start(
                out=blob[:, C_CSL : C_CSL + 4096], in_=d_blob[:, C_CSL : C_CSL + 4096]
            )
            nc.gpsimd.dma_start(
                out=blob[:, C_CSL + 4096 : C_WDD],
                in_=d_blob[:, C_CSL + 4096 : C_WDD],
            )
            nc.scalar.dma_start(out=blob[:, C_WDD:], in_=d_blob[:, C_WDD:])
            row1 = consts.tile([1, 704], bf16)
            nc.gpsimd.dma_start(out=row1, in_=d_row1[:, :])
            ones_sb = consts.tile([1, BCH], bf16)
            nc.vector.memset(ones_sb, 1.0)

            wf = consts.tile([128, 13 * 1024], bf16)
            nc.sync.dma_start(out=wf[:, : 5 * 1024], in_=d_wf[:, : 5 * 1024])
            nc.scalar.dma_start(
                out=wf[:, 5 * 1024 : 9 * 1024], in_=d_wf[:, 5 * 1024 : 9 * 1024]
            )
            nc.gpsimd.dma_start(out=wf[:, 9 * 1024 :], in_=d_wf[:, 9 * 1024 :])
            wa = consts.tile([56, 4 * 1024], bf16)
            nc.sync.dma_start(out=wa, in_=d_wa[:, :])
            wb = consts.tile([120, 4 * 1024], bf16)
            nc.scalar.dma_start(out=wb, in_=d_wb[:, :])
            wtile = {"f": wf, "a": wa, "b": wb}

            st = {}
            # ---- phase A ----
            for bc in range(2):
                b0 = bc * BCH
                ctl = blob[:, C_CT + b0 : C_CT + b0 + BCH]
                cb8 = blob[:, C_CB8 + bc * 1024 : C_CB8 + (bc + 1) * 1024]

                # extras: wamp | bias | 2cg | dotbd
                pex = extp.tile([BCH, 4 * NLOC], f32, name="pex", tag="pex")
                nc.tensor.matmul(
                    pex[:, : 3 * NLOC],
                    ctl,
                    blob[:, C_EW : C_EW + 192],
                    start=True,
                    stop=False,
                )
                nc.tensor.matmul(
                    pex[:, : 3 * NLOC],
                    ones_sb,
                    row1[:, 512:704],
                    start=False,
                    stop=True,
                )
                # diag block
                pdg = miscp.tile([BCH, NLOC * FIN], f32, name="pdg", tag="pdg")
                nc.tensor.matmul(
                    pdg, ctl, blob[:, C_WDD : C_WDD + 512], start=True, stop=False
                )
                nc.tensor.matmul(pdg, ones_sb, row1[:, :512], start=False, stop=True)

                P = big.tile([BCH, NLOC * 128], bf16, name="P", tag="P")
                Q = big.tile([BCH, NLOC * 128], bf16, name="Q", tag="Q")
                for j in range(8):
                    c0 = j * 1024
                    # chol matmul first (S consumes while R matmuls run)
                    spj = sp.tile([BCH, 1024], f32, name="spj", tag="spj")
                    for h in range(2):
                        nc.tensor.matmul(
                            spj[:, h * 512 : (h + 1) * 512],
                            ctl,
                            blob[
                                :, C_CSL + c0 + h * 512 : C_CSL + c0 + (h + 1) * 512
                            ],
                            start=True,
                            stop=True,
                        )
                    rpj = rp.tile([BCH, 1024], f32, name="rpj", tag="rpj")
                    nch = len(EXT_OF_J[j])
                    for t, ext in enumerate(EXT_OF_J[j]):
                        kind, off = SLAB_PACK[(j, t)]
                        wt = wtile[kind]
                        xcol = C_XVT + t * 256 + b0
                        for h in range(2):
                            nc.tensor.matmul(
                                rpj[:, h * 512 : (h + 1) * 512],
                                blob[:ext, xcol : xcol + BCH],
                                wt[:ext, off + h * 512 : off + (h + 1) * 512],
                                start=(t == 0),
                                stop=(t == nch - 1),
                            )
                    # S: square the chol output straight out of PSUM
                    nc.scalar.activation(
                        out=Q[:, c0 : c0 + 1024], in_=spj, func=AF.Square
                    )
                    # S copies R to SBUF; V multiplies at 2x
                    rsj = rsb.tile([BCH, 1024], bf16, name="rsj", tag="rsj")
                    nc.scalar.activation(out=rsj, in_=rpj, func=AF.Copy)
                    nc.vector.tensor_mul(P[:, c0 : c0 + 1024], rsj, cb8)

                # dotbd into pex[:, 192:256]
                for kc in range(4):
                    xcol = C_XVT + kc * 256 + b0
                    nc.tensor.matmul(
                        pex[:, 3 * NLOC :],
                        blob[:, xcol : xcol + BCH],
                        blob[:, C_BDM + kc * 64 : C_BDM + (kc + 1) * 64],
                        start=(kc == 0),
                        stop=(kc == 3),
                    )

                # dot add-tree on V (128 -> 1 per row, last two levels f32)
                DOTL = accs.tile([BCH, NLOC], f32, name="DOTL", tag="DOTL")
                cur = P
                w = 128
                while w > 4:
                    w //= 2
                    nxt = tree.tile([BCH, NLOC * w], bf16, name=f"tr{w}", tag=f"tr{w}")
                    nc.vector.tensor_add(
                        nxt.rearrange("p (r k) -> p r k", k=w),
                        cur.rearrange("p (r k) -> p r k", k=2 * w)[:, :, :w],
                        cur.rearrange("p (r k) -> p r k", k=2 * w)[:, :, w:],
                    )
                    cur = nxt
                t2 = tree.tile([BCH, NLOC * 2], f32, name="tr2", tag="tr2")
                nc.vector.tensor_add(
                    t2.rearrange("p (r k) -> p r k", k=2),
                    cur.rearrange("p (r k) -> p r k", k=4)[:, :, :2],
                    cur.rearrange("p (r k) -> p r k", k=4)[:, :, 2:],
                )
                nc.vector.tensor_add(
                    DOTL,
                    t2.rearrange("p (r k) -> p r k", k=2)[:, :, 0],
                    t2.rearrange("p (r k) -> p r k", k=2)[:, :, 1],
                )
                # square add-tree on V
                SQL = accs.tile([BCH, NLOC], f32, name="SQL", tag="SQL")
                cur = Q
                w = 128
                while w > 4:
                    w //= 2
                    nxt = tree.tile([BCH, NLOC * w], bf16, name=f"sq{w}", tag=f"sq{w}")
                    nc.vector.tensor_add(
                        nxt.rearrange("p (r k) -> p r k", k=w),
                        cur.rearrange("p (r k) -> p r k", k=2 * w)[:, :, :w],
                        cur.rearrange("p (r k) -> p r k", k=2 * w)[:, :, w:],
                    )
                    cur = nxt
                q2 = tree.tile([BCH, NLOC * 2], f32, name="sq2l", tag="sq2l")
                nc.vector.tensor_add(
                    q2.rearrange("p (r k) -> p r k", k=2),
                    cur.rearrange("p (r k) -> p r k", k=4)[:, :, :2],
                    cur.rearrange("p (r k) -> p r k", k=4)[:, :, 2:],
                )
                nc.vector.tensor_add(
                    SQL,
                    q2.rearrange("p (r k) -> p r k", k=2)[:, :, 0],
                    q2.rearrange("p (r k) -> p r k", k=2)[:, :, 1],
                )
                st[bc] = dict(pex=pex, pdg=pdg, DOTL=DOTL, SQL=SQL)

            # ---- phase B: diag elementwise ----
            for bc in range(2):
                s_ = st[bc]
                pdg = s_["pdg"]
                xvd = blob[:, C_XVD + bc * 512 : C_XVD + (bc + 1) * 512]
                xle = blob[:, C_XLE + bc * 512 : C_XLE + (bc + 1) * 512]
                expd = tree.tile([BCH, NLOC * FIN], bf16, name="expd", tag="expd")
                nc.scalar.activation(out=expd, in_=pdg, func=AF.Exp)
                sq2 = tree.tile([BCH, NLOC * FIN], bf16, name="sq2", tag="sq2")
                nc.scalar.activation(out=sq2, in_=pdg, func=AF.Exp, scale=2.0)
                SQD = accs.tile([BCH, NLOC], f32, name="SQD", tag="SQD")
                nc.vector.tensor_reduce(
                    out=SQD,
                    in_=sq2.rearrange("p (r f) -> p r f", f=FIN),
                    axis=mybir.AxisListType.X,
                    op=ALU.add,
                )
                prd = tree.tile([BCH, NLOC * FIN], bf16, name="prd", tag="prd")
                nc.vector.tensor_mul(prd, expd, xvd)
                DOTD = accs.tile([BCH, NLOC], f32, name="DOTD", tag="DOTD")
                nc.vector.tensor_reduce(
                    out=DOTD,
                    in_=prd.rearrange("p (r f) -> p r f", f=FIN),
                    axis=mybir.AxisListType.X,
                    op=ALU.add,
                )
                prl = tree.tile([BCH, NLOC * FIN], bf16, name="prl", tag="prl")
                nc.vector.tensor_mul(prl, expd, xle)
                LDS = accs.tile([BCH, NLOC], f32, name="LDS", tag="LDS")
                nc.vector.tensor_reduce(
                    out=LDS,
                    in_=prl.rearrange("p (r f) -> p r f", f=FIN),
                    axis=mybir.AxisListType.X,
                    op=ALU.add,
                )
                s_.update(SQD=SQD, DOTD=DOTD, LDS=LDS)

            # ---- phase C: assembly ----
            for bc in range(2):
                s_ = st[bc]
                sqn = accs.tile([BCH, NLOC], f32, name="sqn", tag="sqn")
                nc.vector.tensor_add(sqn, s_["SQL"], s_["SQD"])
                nc.vector.tensor_add(sqn, sqn, s_["pex"][:, 2 * NLOC : 3 * NLOC])
                dot = accs.tile([BCH, NLOC], f32, name="dot", tag="dot")
                nc.vector.tensor_add(dot, s_["DOTL"], s_["DOTD"])
                nc.vector.tensor_add(dot, dot, s_["pex"][:, 3 * NLOC :])
                s_.update(sqn=sqn, dot=dot)
            for bc in range(2):
                s_ = st[bc]
                l1 = accs.tile([BCH, NLOC], f32, name="l1", tag="l1")
                nc.scalar.activation(out=l1, in_=s_["sqn"], func=AF.Ln)
                l2 = accs.tile([BCH, NLOC], f32, name="l2", tag="l2")
                nc.scalar.activation(out=l2, in_=s_["LDS"], func=AF.Ln)
                s_.update(l1=l1, l2=l2)
            for bc in range(2):
                s_ = st[bc]
                m1 = accs.tile([BCH, NLOC], f32, name="m1", tag="m1")
                nc.scalar.mul(m1, s_["l1"], -0.5)
                u = accs.tile([BCH, NLOC], f32, name="u", tag="u")
                nc.vector.tensor_add(u, s_["pex"][:, :NLOC], m1)
                s_.update(u=u)
            for bc in range(2):
                s_ = st[bc]
                sc = accs.tile([BCH, NLOC], f32, name="sc", tag="sc")
                nc.scalar.activation(out=sc, in_=s_["u"], func=AF.Exp)
                s_.update(sc=sc)
            for bc in range(2):
                b0 = bc * BCH
                s_ = st[bc]
                yv = accs.tile([BCH, NLOC], f32, name="yv", tag="yv")
                nc.vector.tensor_mul(yv, s_["dot"], s_["sc"])
                yb = accs.tile([BCH, NLOC], f32, name="yb", tag="yb")
                nc.vector.tensor_add(yb, yv, s_["pex"][:, NLOC : 2 * NLOC])
                ld = accs.tile([BCH, NLOC], f32, name="ld", tag="ld")
                nc.vector.tensor_add(ld, s_["u"], s_["l2"])
                ob = accs.tile([BCH, NLOC, 2], f32, name="ob", tag="ob")
                nc.gpsimd.tensor_copy(out=ob[:, :, 0], in_=yb)
                nc.gpsimd.tensor_copy(out=ob[:, :, 1], in_=ld)
                nc.sync.dma_start(out=d_out[b0 : b0 + BCH, :, :], in_=ob)

    nc.compile()
    return nc


def _host_prep(x, c, Wd, bd, Wa, ba, Wb, bb):
    """Build the 8 per-core input maps."""
    import ml_dtypes

    bf = ml_dtypes.bfloat16
    x = np.ascontiguousarray(x, dtype=np.float32)
    c = np.ascontiguousarray(c, dtype=np.float32)
    Wd5 = np.ascontiguousarray(Wd, dtype=np.float32).reshape(CDIM, NCH, FOUT, NCH, FIN)
    bd4 = np.ascontiguousarray(bd, dtype=np.float32).reshape(NCH, FOUT, NCH, FIN)
    Wa_ = np.ascontiguousarray(Wa, dtype=np.float32)
    Wb_ = np.ascontiguousarray(Wb, dtype=np.float32)
    ba = np.ascontiguousarray(ba, dtype=np.float32)
    bb = np.ascontiguousarray(bb, dtype=np.float32)

    cT = np.ascontiguousarray(c.T)
    xv = np.ascontiguousarray(x[:, :, 0])
    xl = np.ascontiguousarray(x[:, :, 1])
    xvT = np.ascontiguousarray(xv.T)

    in_maps = []
    for k in range(NCORES):
        chs = _channels(k)
        blob = np.zeros((CDIM, NCOL), dtype=np.float32)
        row1 = np.zeros((1, 704), dtype=np.float32)
        wfp = np.zeros((128, 13 * 1024), dtype=np.float32)
        wap = np.zeros((56, 4 * 1024), dtype=np.float32)
        wbp = np.zeros((120, 4 * 1024), dtype=np.float32)
        packs = {"f": wfp, "a": wap, "b": wbp}

        blob[:, C_CT : C_CT + 256] = cT
        for t in range(4):
            blob[:, C_XVT + t * 256 : C_XVT + (t + 1) * 256] = xvT[
                t * 128 : (t + 1) * 128, :
            ]
        for bc in range(2):
            blob[:, C_CB8 + bc * 1024 : C_CB8 + (bc + 1) * 1024] = np.tile(
                c[bc * BCH : (bc + 1) * BCH, :], (1, 8)
            )

        bdm = np.zeros((I, NLOC), dtype=np.float32)
        for j, ch in enumerate(chs):
            w = ch * FIN
            Gall = Wd5[:, ch, :, :, :].reshape(CDIM, FOUT, I)  # [128, 8, 512]
            for t, ext in enumerate(EXT_OF_J[j]):
                i0 = 128 * t
                n = max(0, min(w - i0, ext))
                if n > 0:
                    kind, off = SLAB_PACK[(j, t)]
                    seg = Gall[:, :, i0 : i0 + n]  # [128, 8, n]
                    packs[kind][:n, off : off + 1024] = seg.transpose(2, 1, 0).reshape(
                        n, 8 * 128
                    )
            for fo in range(FOUT):
                r = j * FOUT + fo
                G = Gall[:, fo, :w]  # [128, w]
                if w > 0:
                    if w <= 128:
                        blob[:, C_CSL + r * 128 : C_CSL + r * 128 + w] = G
                    else:
                        M = G.astype(np.float64) @ G.astype(np.float64).T
                        M += np.eye(CDIM) * (1e-12 * np.trace(M) / CDIM)
                        blob[:, C_CSL + r * 128 : C_CSL + (r + 1) * 128] = (
                            np.linalg.cholesky(M)
                        )
                blob[:, C_WDD + r * FIN : C_WDD + (r + 1) * FIN] = Wd5[:, ch, fo, ch, :]
                row1[0, r * FIN : (r + 1) * FIN] = bd4[ch, fo, ch, :]
                bd_low = bd4[ch, fo, :ch, :].reshape(-1)
                bdm[:w, r] = bd_low
                if w > 0:
                    blob[:, C_EW + 128 + r] = 2.0 * (G @ bd_low)
                    row1[0, 512 + 128 + r] = np.dot(bd_low, bd_low)
            rows = slice(ch * FOUT, (ch + 1) * FOUT)
            blob[:, C_EW + j * FOUT : C_EW + (j + 1) * FOUT] = Wa_[:, rows]
            blob[:, C_EW + 64 + j * FOUT : C_EW + 64 + (j + 1) * FOUT] = Wb_[:, rows]
            row1[0, 512 + j * FOUT : 512 + (j + 1) * FOUT] = ba[rows]
            row1[0, 512 + 64 + j * FOUT : 512 + 64 + (j + 1) * FOUT] = bb[rows]

        for kc in range(4):
            blob[:, C_BDM + kc * 64 : C_BDM + (kc + 1) * 64] = bdm[
                kc * 128 : (kc + 1) * 128, :
            ]

        xvd = np.empty((B, NLOC * FIN), dtype=np.float32)
        xled = np.empty((B, NLOC * FIN), dtype=np.float32)
        for r in range(NLOC):
            ch = chs[r // FOUT]
            xvd[:, r * FIN : (r + 1) * FIN] = xv[:, ch * FIN : (ch + 1) * FIN]
            xled[:, r * FIN : (r + 1) * FIN] = xl[:, ch * FIN : (ch + 1) * FIN]
        np.exp(xled, out=xled)
        for bc in range(2):
            blob[:, C_XVD + bc * 512 : C_XVD + (bc + 1) * 512] = xvd[
                bc * BCH : (bc + 1) * BCH, :
            ]
            blob[:, C_XLE + bc * 512 : C_XLE + (bc + 1) * 512] = xled[
                bc * BCH : (bc + 1) * BCH, :
            ]

        in_maps.append(
            {
                "blob": blob.astype(bf),
                "row1": row1.astype(bf),
                "wf": wfp.astype(bf),
                "wa": wap.astype(bf),
                "wb": wbp.astype(bf),
            }
        )
    return in_maps


def kernel(x, c, Wd, bd, Wa, ba, Wb, bb, _trace=False, _tmpdir=None):
    global _NC
    from concourse.bass_utils import run_bass_kernel_spmd

    if _NC is None:
        _NC = _build_nc()
    in_maps = _host_prep(x, c, Wd, bd, Wa, ba, Wb, bb)
    res = run_bass_kernel_spmd(
        _NC, in_maps, core_ids=list(range(NCORES)), trace=_trace, tmpdir=_tmpdir
    )

    out = np.empty((B, O, 2), dtype=np.float32)
    for k in range(NCORES):
        ok = res.results[k]["out"]
        for j, ch in enumerate(_channels(k)):
            out[:, ch * FOUT : (ch + 1) * FOUT, :] = ok[:, j * FOUT : (j + 1) * FOUT, :]
    if _trace:
        return out, res
    return out


# revision 11
# speedup vs baseline: 1.3658x; 1.0077x over previous
"""Trainium2 Bass kernel for nn_CLinear_6768868459230.

Context-conditioned block-autoregressive linear layer (MAF-style):
  wdir = c @ Wd + bd                      [B, O, I]
  w    = exp(wdir)*mask_diag + wdir*mask_lower
  sqn  = sum(w^2, axis=I)
  y    = (w / sqrt(sqn) * exp(wamp)) @ xv + bias
  logdet = logsumexp over diag block of (wdir - 0.5 log sqn + wamp + xl)

Sharding: tensor-parallel over the O=512 output rows; core k owns channels
{k, 15-k, 16+k, 31-k, ...} so the triangular work is the same on every core.

Algorithm (v3): never materialize t = c @ Wd_lower (whose [B, ~17k]
elementwise reductions bound the baseline).  Instead per output row o:
  dot_lower = c^T (W_o x)   -> R[b,(o,k)] = sum_i W[k,o,i] x[b,i] on
     TensorE (i-chunked, true-K, no padding cost), then one multiply by c
     and a bf16 add-tree: 128 terms/row instead of up to 504.
  sqn_lower = |C_o^T c|^2 + 2(G_o bd)^T c + |bd|^2, C_o = G_o (w<=128) or
     chol(G_o G_o^T): ScalarE squares the 128-wide s = C^T c straight out
     of PSUM (bf16 out), GpSimd segment-reduces.
Engine split per 128-sample chunk: TensorE ~30k cols; PSUM evacuation
split S (squares + 4 slot copies) / V (4 direct-PSUM mults); V runs the
bf16 dot-tree; GpSimd the square-reduce, diag products and output
interleave.  All operands bf16; DMAs consolidated into ~14 transfers.
"""

import numpy as np

NCH, FIN, FOUT, CDIM, B = 64, 8, 8, 128, 256
I = NCH * FIN
O = NCH * FOUT
NCORES = 8
NLOC = 64  # output rows per core
BCH = 128  # batch chunk (SBUF partitions)

# per-slot i-chunk extents (max over cores of w=8*ch for that slot)
EXT_OF_J = [
    [56],
    [120],
    [128, 56],
    [128, 120],
    [128, 128, 56],
    [128, 128, 120],
    [128, 128, 128, 56],
    [128, 128, 128, 120],
]
# slab packing: full 128-row slabs in one [128, 13*1024] tensor; the
# 56-row and 120-row tails in [56, 4*1024] and [120, 4*1024] tensors.
SLAB_PACK = {}
_nf = _n56 = _n120 = 0
for _j in range(8):
    for _t, _e in enumerate(EXT_OF_J[_j]):
        if _e == 128:
            SLAB_PACK[(_j, _t)] = ("f", _nf * 1024)
            _nf += 1
        elif _e == 56:
            SLAB_PACK[(_j, _t)] = ("a", _n56 * 1024)
            _n56 += 1
        else:
            SLAB_PACK[(_j, _t)] = ("b", _n120 * 1024)
            _n120 += 1

# col offsets inside the consolidated [128, NCOL] blob
C_CT = 0          # [128, 256]   c transposed
C_XVT = 256       # [128, 4*256] xv transposed, 4 i-chunks
C_CB8 = 1280      # [128, 2*1024] c tiled 8x per batch chunk
C_CSL = 3328      # [128, 64*128] chol/G factors
C_WDD = 11520     # [128, 512]   diag weights
C_BDM = 12032     # [128, 4*64]  bd-lower matvec weights, 4 i-chunks
C_XVD = 12288     # [128, 2*512] diag-gathered xv per batch chunk
C_XLE = 13312     # [128, 2*512] diag-gathered exp(xl)
C_EW = 14336      # [128, 192]   wamp | bias | 2cg weights
NCOL = 14528


def _channels(k):
    return [k, 15 - k, 16 + k, 31 - k, 32 + k, 47 - k, 48 + k, 63 - k]


_NC = None


def _build_nc():
    import concourse.bacc as bacc
    import concourse.tile as tile
    from concourse import mybir

    f32 = mybir.dt.float32
    bf16 = mybir.dt.bfloat16
    AF = mybir.ActivationFunctionType
    ALU = mybir.AluOpType

    nc = bacc.Bacc(None, target_bir_lowering=False)

    d_blob = nc.dram_tensor("blob", [CDIM, NCOL], bf16, kind="ExternalInput")
    d_row1 = nc.dram_tensor("row1", [1, 704], bf16, kind="ExternalInput")
    d_wf = nc.dram_tensor("wf", [128, 13 * 1024], bf16, kind="ExternalInput")
    d_wa = nc.dram_tensor("wa", [56, 4 * 1024], bf16, kind="ExternalInput")
    d_wb = nc.dram_tensor("wb", [120, 4 * 1024], bf16, kind="ExternalInput")
    d_out = nc.dram_tensor("out", [B, NLOC, 2], f32, kind="ExternalOutput")

    with tile.TileContext(nc) as tc:
        with (
            tc.tile_pool(name="consts", bufs=1) as consts,
            tc.tile_pool(name="rsb", bufs=3) as rsb,
            tc.tile_pool(name="big", bufs=2) as big,
            tc.tile_pool(name="tree", bufs=1) as tree,
            tc.tile_pool(name="accs", bufs=2) as accs,
            tc.tile_pool(name="rp", bufs=2, space="PSUM") as rp,
            tc.tile_pool(name="sp", bufs=1, space="PSUM") as sp,
            tc.tile_pool(name="miscp", bufs=1, space="PSUM") as miscp,
            tc.tile_pool(name="extp", bufs=1, space="PSUM") as extp,
        ):
            blob = consts.tile([CDIM, NCOL], bf16)
            # first-used columns first, spread over the three DGE queues
            nc.sync.dma_start(out=blob[:, :C_CSL], in_=d_blob[:, :C_CSL])
            nc.scalar.dma_start(
                out=blob[:, C_CSL : C_CSL + 4096], in_=d_blob[:, C_CSL : C_CSL + 4096]
            )
            nc.gpsimd.dma_start(
                out=blob[:, C_CSL + 4096 : C_WDD],
                in_=d_blob[:, C_CSL + 4096 : C_WDD],
            )
            nc.scalar.dma_start(out=blob[:, C_WDD:], in_=d_blob[:, C_WDD:])
            row1 = consts.tile([1, 704], bf16)
            nc.gpsimd.dma_start(out=row1, in_=d_row1[:, :])
            ones_sb = consts.tile([1, BCH], bf16)
            nc.vector.memset(ones_sb, 1.0)

            wf = consts.tile([128, 13 * 1024], bf16)
            nc.sync.dma_start(out=wf[:, : 5 * 1024], in_=d_wf[:, : 5 * 1024])
            nc.scalar.dma_start(
                out=wf[:, 5 * 1024 : 9 * 1024], in_=d_wf[:, 5 * 1024 : 9 * 1024]
            )
            nc.gpsimd.dma_start(out=wf[:, 9 * 1024 :], in_=d_wf[:, 9 * 1024 :])
            wa = consts.tile([56, 4 * 1024], bf16)
            nc.sync.dma_start(out=wa, in_=d_wa[:, :])
            wb = consts.tile([120, 4 * 1024], bf16)
            nc.scalar.dma_start(out=wb, in_=d_wb[:, :])
            wtile = {"f": wf, "a": wa, "b": wb}

            st = {}
            # ---- phase A ----
            for bc in range(2):
                b0 = bc * BCH
                ctl = blob[:, C_CT + b0 : C_CT + b0 + BCH]
                cb8 = blob[:, C_CB8 + bc * 1024 : C_CB8 + (bc + 1) * 1024]

                # extras: wamp | bias | 2cg | dotbd
                pex = extp.tile([BCH, 4 * NLOC], f32, name="pex", tag="pex")
                nc.tensor.matmul(
                    pex[:, : 3 * NLOC],
                    ctl,
                    blob[:, C_EW : C_EW + 192],
                    start=True,
                    stop=False,
                )
                nc.tensor.matmul(
                    pex[:, : 3 * NLOC],
                    ones_sb,
                    row1[:, 512:704],
                    start=False,
                    stop=True,
                )
                # diag block
                pdg = miscp.tile([BCH, NLOC * FIN], f32, name="pdg", tag="pdg")
                nc.tensor.matmul(
                    pdg, ctl, blob[:, C_WDD : C_WDD + 512], start=True, stop=False
                )
                nc.tensor.matmul(pdg, ones_sb, row1[:, :512], start=False, stop=True)

                P = big.tile([BCH, NLOC * 128], bf16, name="P", tag="P")
                Q = big.tile([BCH, NLOC * 128], bf16, name="Q", tag="Q")
                for j in range(8):
                    c0 = j * 1024
                    # chol matmul first (S consumes while R matmuls run)
                    spj = sp.tile([BCH, 1024], f32, name="spj", tag="spj")
                    for h in range(2):
                        nc.tensor.matmul(
                            spj[:, h * 512 : (h + 1) * 512],
                            ctl,
                            blob[
                                :, C_CSL + c0 + h * 512 : C_CSL + c0 + (h + 1) * 512
                            ],
                            start=True,
                            stop=True,
                        )
                    rpj = rp.tile([BCH, 1024], f32, name="rpj", tag="rpj")
                    nch = len(EXT_OF_J[j])
                    for t, ext in enumerate(EXT_OF_J[j]):
                        kind, off = SLAB_PACK[(j, t)]
                        wt = wtile[kind]
                        xcol = C_XVT + t * 256 + b0
                        for h in range(2):
                            nc.tensor.matmul(
                                rpj[:, h * 512 : (h + 1) * 512],
                                blob[:ext, xcol : xcol + BCH],
                                wt[:ext, off + h * 512 : off + (h + 1) * 512],
                                start=(t == 0),
                                stop=(t == nch - 1),
                            )
                    # S: square the chol output straight out of PSUM
                    nc.scalar.activation(
                        out=Q[:, c0 : c0 + 1024], in_=spj, func=AF.Square
                    )
                    if j < 6:
                        # S copies R to SBUF; V multiplies at 2x
                        rsj = rsb.tile([BCH, 1024], bf16, name="rsj", tag="rsj")
                        nc.scalar.activation(out=rsj, in_=rpj, func=AF.Copy)
                        nc.vector.tensor_mul(P[:, c0 : c0 + 1024], rsj, cb8)
                    else:
                        # V multiplies straight out of PSUM
                        nc.vector.tensor_mul(P[:, c0 : c0 + 1024], rpj, cb8)

                # dotbd into pex[:, 192:256]
                for kc in range(4):
                    xcol = C_XVT + kc * 256 + b0
                    nc.tensor.matmul(
                        pex[:, 3 * NLOC :],
                        blob[:, xcol : xcol + BCH],
                        blob[:, C_BDM + kc * 64 : C_BDM + (kc + 1) * 64],
                        start=(kc == 0),
                        stop=(kc == 3),
                    )

                # add-trees on V, per half (rows 0-31 start after slot 3)
                DOTL = accs.tile([BCH, NLOC], f32, name="DOTL", tag="DOTL")
                SQL = accs.tile([BCH, NLOC], f32, name="SQL", tag="SQL")
                HR = NLOC // 2
                for hh, (src_, dst) in enumerate(
                    ((P, DOTL), (Q, SQL), (P, DOTL), (Q, SQL))
                ):
                    half = hh // 2
                    r0 = half * HR
                    nm = ("d", "s")[hh % 2] + str(half)
                    cur = src_[:, r0 * 128 : (r0 + HR) * 128]
                    w = 128
                    while w > 4:
                        w //= 2
                        nxt = tree.tile(
                            [BCH, HR * w], bf16, name=f"t{nm}{w}", tag=f"t{nm}{w}"
                        )
                        nc.vector.tensor_add(
                            nxt.rearrange("p (r k) -> p r k", k=w),
                            cur.rearrange("p (r k) -> p r k", k=2 * w)[:, :, :w],
                            cur.rearrange("p (r k) -> p r k", k=2 * w)[:, :, w:],
                        )
                        cur = nxt
                    t2 = tree.tile([BCH, HR * 2], f32, name=f"t2{nm}", tag=f"t2{nm}")
                    nc.vector.tensor_add(
                        t2.rearrange("p (r k) -> p r k", k=2),
                        cur.rearrange("p (r k) -> p r k", k=4)[:, :, :2],
                        cur.rearrange("p (r k) -> p r k", k=4)[:, :, 2:],
                    )
                    nc.vector.tensor_add(
                        dst[:, r0 : r0 + HR],
                        t2.rearrange("p (r k) -> p r k", k=2)[:, :, 0],
                        t2.rearrange("p (r k) -> p r k", k=2)[:, :, 1],
                    )
                st[bc] = dict(pex=pex, pdg=pdg, DOTL=DOTL, SQL=SQL)

            # ---- phase B: diag elementwise ----
            for bc in range(2):
                s_ = st[bc]
                pdg = s_["pdg"]
                xvd = blob[:, C_XVD + bc * 512 : C_XVD + (bc + 1) * 512]
                xle = blob[:, C_XLE + bc * 512 : C_XLE + (bc + 1) * 512]
                expd = tree.tile([BCH, NLOC * FIN], bf16, name="expd", tag="expd")
                nc.scalar.activation(out=expd, in_=pdg, func=AF.Exp)
                sq2 = tree.tile([BCH, NLOC * FIN], bf16, name="sq2", tag="sq2")
                nc.scalar.activation(out=sq2, in_=pdg, func=AF.Exp, scale=2.0)
                SQD = accs.tile([BCH, NLOC], f32, name="SQD", tag="SQD")
                nc.vector.tensor_reduce(
                    out=SQD,
                    in_=sq2.rearrange("p (r f) -> p r f", f=FIN),
                    axis=mybir.AxisListType.X,
                    op=ALU.add,
                )
                prd = tree.tile([BCH, NLOC * FIN], bf16, name="prd", tag="prd")
                nc.vector.tensor_mul(prd, expd, xvd)
                DOTD = accs.tile([BCH, NLOC], f32, name="DOTD", tag="DOTD")
                nc.vector.tensor_reduce(
                    out=DOTD,
                    in_=prd.rearrange("p (r f) -> p r f", f=FIN),
                    axis=mybir.AxisListType.X,
                    op=ALU.add,
                )
                prl = tree.tile([BCH, NLOC * FIN], bf16, name="prl", tag="prl")
                nc.vector.tensor_mul(prl, expd, xle)
                LDS = accs.tile([BCH, NLOC], f32, name="LDS", tag="LDS")
                nc.vector.tensor_reduce(
                    out=LDS,
                    in_=prl.rearrange("p (r f) -> p r f", f=FIN),
                    axis=mybir.AxisListType.X,
                    op=ALU.add,
                )
                s_.update(SQD=SQD, DOTD=DOTD, LDS=LDS)

            # ---- phase C: assembly ----
            for bc in range(2):
                s_ = st[bc]
                sqn = accs.tile([BCH, NLOC], f32, name="sqn", tag="sqn")
                nc.vector.tensor_add(sqn, s_["SQL"], s_["SQD"])
                nc.vector.tensor_add(sqn, sqn, s_["pex"][:, 2 * NLOC : 3 * NLOC])
                dot = accs.tile([BCH, NLOC], f32, name="dot", tag="dot")
                nc.vector.tensor_add(dot, s_["DOTL"], s_["DOTD"])
                nc.vector.tensor_add(dot, dot, s_["pex"][:, 3 * NLOC :])
                s_.update(sqn=sqn, dot=dot)
            for bc in range(2):
                s_ = st[bc]
                l1 = accs.tile([BCH, NLOC], f32, name="l1", tag="l1")
                nc.scalar.activation(out=l1, in_=s_["sqn"], func=AF.Ln)
                l2 = accs.tile([BCH, NLOC], f32, name="l2", tag="l2")
                nc.scalar.activation(out=l2, in_=s_["LDS"], func=AF.Ln)
                s_.update(l1=l1, l2=l2)
            for bc in range(2):
                s_ = st[bc]
                m1 = accs.tile([BCH, NLOC], f32, name="m1", tag="m1")
                nc.scalar.mul(m1, s_["l1"], -0.5)
                u = accs.tile([BCH, NLOC], f32, name="u", tag="u")
                nc.vector.tensor_add(u, s_["pex"][:, :NLOC], m1)
                s_.update(u=u)
            for bc in range(2):
                s_ = st[bc]
                sc = accs.tile([BCH, NLOC], f32, name="sc", tag="sc")
                nc.scalar.activation(out=sc, in_=s_["u"], func=AF.Exp)
                s_.update(sc=sc)
            for bc in range(2):
                b0 = bc * BCH
                s_ = st[bc]
                yv = accs.tile([BCH, NLOC], f32, name="yv", tag="yv")
                nc.vector.tensor_mul(yv, s_["dot"], s_["sc"])
                yb = accs.tile([BCH, NLOC], f32, name="yb", tag="yb")
                nc.vector.tensor_add(yb, yv, s_["pex"][:, NLOC : 2 * NLOC])
                ld = accs.tile([BCH, NLOC], f32, name="ld", tag="ld")
                nc.vector.tensor_add(ld, s_["u"], s_["l2"])
                ob = accs.tile([BCH, NLOC, 2], f32, name="ob", tag="ob")
                nc.gpsimd.tensor_copy(out=ob[:, :, 0], in_=yb)
                nc.gpsimd.tensor_copy(out=ob[:, :, 1], in_=ld)
                nc.sync.dma_start(out=d_out[b0 : b0 + BCH, :, :], in_=ob)

    nc.compile()
    return nc


def _host_prep(x, c, Wd, bd, Wa, ba, Wb, bb):
    """Build the 8 per-core input maps."""
    import ml_dtypes

    bf = ml_dtypes.bfloat16
    x = np.ascontiguousarray(x, dtype=np.float32)
    c = np.ascontiguousarray(c, dtype=np.float32)
    Wd5 = np.ascontiguousarray(Wd, dtype=np.float32).reshape(CDIM, NCH, FOUT, NCH, FIN)
    bd4 = np.ascontiguousarray(bd, dtype=np.float32).reshape(NCH, FOUT, NCH, FIN)
    Wa_ = np.ascontiguousarray(Wa, dtype=np.float32)
    Wb_ = np.ascontiguousarray(Wb, dtype=np.float32)
    ba = np.ascontiguousarray(ba, dtype=np.float32)
    bb = np.ascontiguousarray(bb, dtype=np.float32)

    cT = np.ascontiguousarray(c.T)
    xv = np.ascontiguousarray(x[:, :, 0])
    xl = np.ascontiguousarray(x[:, :, 1])
    xvT = np.ascontiguousarray(xv.T)

    in_maps = []
    for k in range(NCORES):
        chs = _channels(k)
        blob = np.zeros((CDIM, NCOL), dtype=np.float32)
        row1 = np.zeros((1, 704), dtype=np.float32)
        wfp = np.zeros((128, 13 * 1024), dtype=np.float32)
        wap = np.zeros((56, 4 * 1024), dtype=np.float32)
        wbp = np.zeros((120, 4 * 1024), dtype=np.float32)
        packs = {"f": wfp, "a": wap, "b": wbp}

        blob[:, C_CT : C_CT + 256] = cT
        for t in range(4):
            blob[:, C_XVT + t * 256 : C_XVT + (t + 1) * 256] = xvT[
                t * 128 : (t + 1) * 128, :
            ]
        for bc in range(2):
            blob[:, C_CB8 + bc * 1024 : C_CB8 + (bc + 1) * 1024] = np.tile(
                c[bc * BCH : (bc + 1) * BCH, :], (1, 8)
            )

        bdm = np.zeros((I, NLOC), dtype=np.float32)
        for j, ch in enumerate(chs):
            w = ch * FIN
            Gall = Wd5[:, ch, :, :, :].reshape(CDIM, FOUT, I)  # [128, 8, 512]
            for t, ext in enumerate(EXT_OF_J[j]):
                i0 = 128 * t
                n = max(0, min(w - i0, ext))
                if n > 0:
                    kind, off = SLAB_PACK[(j, t)]
                    seg = Gall[:, :, i0 : i0 + n]  # [128, 8, n]
                    packs[kind][:n, off : off + 1024] = seg.transpose(2, 1, 0).reshape(
                        n, 8 * 128
                    )
            for fo in range(FOUT):
                r = j * FOUT + fo
                G = Gall[:, fo, :w]  # [128, w]
                if w > 0:
                    if w <= 128:
                        blob[:, C_CSL + r * 128 : C_CSL + r * 128 + w] = G
                    else:
                        M = G.astype(np.float64) @ G.astype(np.float64).T
                        M += np.eye(CDIM) * (1e-12 * np.trace(M) / CDIM)
                        blob[:, C_CSL + r * 128 : C_CSL + (r + 1) * 128] = (
                            np.linalg.cholesky(M)
                        )
                blob[:, C_WDD + r * FIN : C_WDD + (r + 1) * FIN] = Wd5[:, ch, fo, ch, :]
                row1[0, r * FIN : (r + 1) * FIN] = bd4[ch, fo, ch, :]
                bd_low = bd4[ch, fo, :ch, :].reshape(-1)
                bdm[:w, r] = bd_low
                if w > 0:
                    blob[:, C_EW + 128 + r] = 2.0 * (G @ bd_low)
                    row1[0, 512 + 128 + r] = np.dot(bd_low, bd_low)
            rows = slice(ch * FOUT, (ch + 1) * FOUT)
            blob[:, C_EW + j * FOUT : C_EW + (j + 1) * FOUT] = Wa_[:, rows]
            blob[:, C_EW + 64 + j * FOUT : C_EW + 64 + (j + 1) * FOUT] = Wb_[:, rows]
            row1[0, 512 + j * FOUT : 512 + (j + 1) * FOUT] = ba[rows]
            row1[0, 512 + 64 + j * FOUT : 512 + 64 + (j + 1) * FOUT] = bb[rows]

        for kc in range(4):
            blob[:, C_BDM + kc * 64 : C_BDM + (kc + 1) * 64] = bdm[
                kc * 128 : (kc + 1) * 128, :
            ]

        xvd = np.empty((B, NLOC * FIN), dtype=np.float32)
        xled = np.empty((B, NLOC * FIN), dtype=np.float32)
        for r in range(NLOC):
            ch = chs[r // FOUT]
            xvd[:, r * FIN : (r + 1) * FIN] = xv[:, ch * FIN : (ch + 1) * FIN]
            xled[:, r * FIN : (r + 1) * FIN] = xl[:, ch * FIN : (ch + 1) * FIN]
        np.exp(xled, out=xled)
        for bc in range(2):
            blob[:, C_XVD + bc * 512 : C_XVD + (bc + 1) * 512] = xvd[
                bc * BCH : (bc + 1) * BCH, :
            ]
            blob[:, C_XLE + bc * 512 : C_XLE + (bc + 1) * 512] = xled[
                bc * BCH : (bc + 1) * BCH, :
            ]

        in_maps.append(
            {
                "blob": blob.astype(bf),
                "row1": row1.astype(bf),
                "wf": wfp.astype(bf),
                "wa": wap.astype(bf),
                "wb": wbp.astype(bf),
            }
        )
    return in_maps


def kernel(x, c, Wd, bd, Wa, ba, Wb, bb, _trace=False, _tmpdir=None):
    global _NC
    from concourse.bass_utils import run_bass_kernel_spmd

    if _NC is None:
        _NC = _build_nc()
    in_maps = _host_prep(x, c, Wd, bd, Wa, ba, Wb, bb)
    res = run_bass_kernel_spmd(
        _NC, in_maps, core_ids=list(range(NCORES)), trace=_trace, tmpdir=_tmpdir
    )

    out = np.empty((B, O, 2), dtype=np.float32)
    for k in range(NCORES):
        ok = res.results[k]["out"]
        for j, ch in enumerate(_channels(k)):
            out[:, ch * FOUT : (ch + 1) * FOUT, :] = ok[:, j * FOUT : (j + 1) * FOUT, :]
    if _trace:
        return out, res
    return out


# revision 13
# speedup vs baseline: 1.4183x; 1.0385x over previous
"""Trainium2 Bass kernel for nn_CLinear_6768868459230.

Context-conditioned block-autoregressive linear layer (MAF-style):
  wdir = c @ Wd + bd                      [B, O, I]
  w    = exp(wdir)*mask_diag + wdir*mask_lower
  sqn  = sum(w^2, axis=I)
  y    = (w / sqrt(sqn) * exp(wamp)) @ xv + bias
  logdet = logsumexp over diag block of (wdir - 0.5 log sqn + wamp + xl)

Sharding: tensor-parallel over the O=512 output rows; core k owns channels
{k, 15-k, 16+k, 31-k, ...} so the triangular work is the same on every core.

Algorithm (v3): never materialize t = c @ Wd_lower (whose [B, ~17k]
elementwise reductions bound the baseline).  Instead per output row o:
  dot_lower = c^T (W_o x)   -> R[b,(o,k)] = sum_i W[k,o,i] x[b,i] on
     TensorE (i-chunked, true-K, no padding cost), then one multiply by c
     and a bf16 add-tree: 128 terms/row instead of up to 504.
  sqn_lower = |C_o^T c|^2 + 2(G_o bd)^T c + |bd|^2, C_o = G_o (w<=128) or
     chol(G_o G_o^T): ScalarE squares the 128-wide s = C^T c straight out
     of PSUM (bf16 out), GpSimd segment-reduces.
Engine split per 128-sample chunk: TensorE ~30k cols; PSUM evacuation
split S (squares + 4 slot copies) / V (4 direct-PSUM mults); V runs the
bf16 dot-tree; GpSimd the square-reduce, diag products and output
interleave.  All operands bf16; DMAs consolidated into ~14 transfers.
"""

import numpy as np

NCH, FIN, FOUT, CDIM, B = 64, 8, 8, 128, 256
I = NCH * FIN
O = NCH * FOUT
NCORES = 8
NLOC = 64  # output rows per core
BCH = 128  # batch chunk (SBUF partitions)

# per-slot i-chunk extents (max over cores of w=8*ch for that slot)
EXT_OF_J = [
    [56],
    [120],
    [128, 56],
    [128, 120],
    [128, 128, 56],
    [128, 128, 120],
    [128, 128, 128, 56],
    [128, 128, 128, 120],
]
# slab packing: full 128-row slabs in one [128, 13*1024] tensor; the
# 56-row and 120-row tails in [56, 4*1024] and [120, 4*1024] tensors.
SLAB_PACK = {}
_nf = _n56 = _n120 = 0
for _j in range(8):
    for _t, _e in enumerate(EXT_OF_J[_j]):
        if _e == 128:
            SLAB_PACK[(_j, _t)] = ("f", _nf * 1024)
            _nf += 1
        elif _e == 56:
            SLAB_PACK[(_j, _t)] = ("a", _n56 * 1024)
            _n56 += 1
        else:
            SLAB_PACK[(_j, _t)] = ("b", _n120 * 1024)
            _n120 += 1

# col offsets inside the consolidated [128, NCOL] blob
C_CT = 0          # [128, 256]   c transposed
C_XVT = 256       # [128, 4*256] xv transposed, 4 i-chunks
C_CB8 = 1280      # [128, 2*1024] c tiled 8x per batch chunk
C_CSL = 3328      # [128, 64*128] chol/G factors
C_WDD = 11520     # [128, 512]   diag weights
C_BDM = 12032     # [128, 4*64]  bd-lower matvec weights, 4 i-chunks
C_XVD = 12288     # [128, 2*512] diag-gathered xv per batch chunk
C_XLE = 13312     # [128, 2*512] diag-gathered exp(xl)
C_EW = 14336      # [128, 192]   wamp | bias | 2cg weights
NCOL = 14528


def _channels(k):
    return [k, 15 - k, 16 + k, 31 - k, 32 + k, 47 - k, 48 + k, 63 - k]


_NC = None


def _build_nc():
    import concourse.bacc as bacc
    import concourse.tile as tile
    from concourse import mybir

    f32 = mybir.dt.float32
    bf16 = mybir.dt.bfloat16
    AF = mybir.ActivationFunctionType
    ALU = mybir.AluOpType

    nc = bacc.Bacc(None, target_bir_lowering=False)

    d_blob = nc.dram_tensor("blob", [CDIM, NCOL], bf16, kind="ExternalInput")
    d_row1 = nc.dram_tensor("row1", [1, 704], bf16, kind="ExternalInput")
    d_wf = nc.dram_tensor("wf", [128, 13 * 1024], bf16, kind="ExternalInput")
    d_wa = nc.dram_tensor("wa", [56, 4 * 1024], bf16, kind="ExternalInput")
    d_wb = nc.dram_tensor("wb", [120, 4 * 1024], bf16, kind="ExternalInput")
    d_out = nc.dram_tensor("out", [B, NLOC, 2], f32, kind="ExternalOutput")

    with tile.TileContext(nc) as tc:
        with (
            tc.tile_pool(name="consts", bufs=1) as consts,
            tc.tile_pool(name="rsb", bufs=3) as rsb,
            tc.tile_pool(name="big", bufs=2) as big,
            tc.tile_pool(name="tree", bufs=1) as tree,
            tc.tile_pool(name="accs", bufs=2) as accs,
            tc.tile_pool(name="rp", bufs=2, space="PSUM") as rp,
            tc.tile_pool(name="sp", bufs=1, space="PSUM") as sp,
            tc.tile_pool(name="miscp", bufs=1, space="PSUM") as miscp,
            tc.tile_pool(name="extp", bufs=1, space="PSUM") as extp,
        ):
            blob = consts.tile([CDIM, NCOL], bf16)
            row1 = consts.tile([1, 704], bf16)
            wf = consts.tile([128, 13 * 1024], bf16)
            wa = consts.tile([56, 4 * 1024], bf16)
            wb = consts.tile([120, 4 * 1024], bf16)
            wtile = {"f": wf, "a": wa, "b": wb}
            ones_sb = consts.tile([1, BCH], bf16)
            nc.vector.memset(ones_sb, 1.0)

            # Stream inputs in compute-use order: small operand blocks first
            # so matmuls start within ~2us, then csl/W slabs slot by slot.
            # sync + scalar queues carry the bulk (HW DGEs); gpsimd carries
            # the small early blocks (its queue is otherwise idle).
            nc.gpsimd.dma_start(out=blob[:, :C_CSL], in_=d_blob[:, :C_CSL])
            nc.gpsimd.dma_start(out=blob[:, C_WDD:], in_=d_blob[:, C_WDD:])
            nc.gpsimd.dma_start(out=row1, in_=d_row1[:, :])
            # per-slot weight arrivals, alternating sync/scalar
            _sq = [nc.sync, nc.scalar]
            _qi = 0

            def _q():
                nonlocal _qi
                q = _sq[_qi % 2]
                _qi += 1
                return q

            for j in range(8):
                _q().dma_start(
                    out=blob[:, C_CSL + j * 1024 : C_CSL + (j + 1) * 1024],
                    in_=d_blob[:, C_CSL + j * 1024 : C_CSL + (j + 1) * 1024],
                )
                for t in range(len(EXT_OF_J[j])):
                    kind, off = SLAB_PACK[(j, t)]
                    wt = wtile[kind]
                    dt = {"f": d_wf, "a": d_wa, "b": d_wb}[kind]
                    _q().dma_start(
                        out=wt[:, off : off + 1024], in_=dt[:, off : off + 1024]
                    )

            st = {}
            # ---- phase A ----
            for bc in range(2):
                b0 = bc * BCH
                ctl = blob[:, C_CT + b0 : C_CT + b0 + BCH]
                cb8 = blob[:, C_CB8 + bc * 1024 : C_CB8 + (bc + 1) * 1024]

                # extras: wamp | bias | 2cg | dotbd
                pex = extp.tile([BCH, 4 * NLOC], f32, name="pex", tag="pex")
                nc.tensor.matmul(
                    pex[:, : 3 * NLOC],
                    ctl,
                    blob[:, C_EW : C_EW + 192],
                    start=True,
                    stop=False,
                )
                nc.tensor.matmul(
                    pex[:, : 3 * NLOC],
                    ones_sb,
                    row1[:, 512:704],
                    start=False,
                    stop=True,
                )
                # diag block
                pdg = miscp.tile([BCH, NLOC * FIN], f32, name="pdg", tag="pdg")
                nc.tensor.matmul(
                    pdg, ctl, blob[:, C_WDD : C_WDD + 512], start=True, stop=False
                )
                nc.tensor.matmul(pdg, ones_sb, row1[:, :512], start=False, stop=True)

                P = big.tile([BCH, NLOC * 128], bf16, name="P", tag="P")
                Q = big.tile([BCH, NLOC * 128], bf16, name="Q", tag="Q")
                for j in range(8):
                    c0 = j * 1024
                    # chol matmul first (S consumes while R matmuls run)
                    spj = sp.tile([BCH, 1024], f32, name="spj", tag="spj")
                    for h in range(2):
                        nc.tensor.matmul(
                            spj[:, h * 512 : (h + 1) * 512],
                            ctl,
                            blob[
                                :, C_CSL + c0 + h * 512 : C_CSL + c0 + (h + 1) * 512
                            ],
                            start=True,
                            stop=True,
                        )
                    rpj = rp.tile([BCH, 1024], f32, name="rpj", tag="rpj")
                    nch = len(EXT_OF_J[j])
                    for t, ext in enumerate(EXT_OF_J[j]):
                        kind, off = SLAB_PACK[(j, t)]
                        wt = wtile[kind]
                        xcol = C_XVT + t * 256 + b0
                        for h in range(2):
                            nc.tensor.matmul(
                                rpj[:, h * 512 : (h + 1) * 512],
                                blob[:ext, xcol : xcol + BCH],
                                wt[:ext, off + h * 512 : off + (h + 1) * 512],
                                start=(t == 0),
                                stop=(t == nch - 1),
                            )
                    # S: square the chol output straight out of PSUM
                    nc.scalar.activation(
                        out=Q[:, c0 : c0 + 1024], in_=spj, func=AF.Square
                    )
                    if j < 6:
                        # S copies R to SBUF; V multiplies at 2x
                        rsj = rsb.tile([BCH, 1024], bf16, name="rsj", tag="rsj")
                        nc.scalar.activation(out=rsj, in_=rpj, func=AF.Copy)
                        nc.vector.tensor_mul(P[:, c0 : c0 + 1024], rsj, cb8)
                    else:
                        # V multiplies straight out of PSUM
                        nc.vector.tensor_mul(P[:, c0 : c0 + 1024], rpj, cb8)

                # dotbd into pex[:, 192:256]
                for kc in range(4):
                    xcol = C_XVT + kc * 256 + b0
                    nc.tensor.matmul(
                        pex[:, 3 * NLOC :],
                        blob[:, xcol : xcol + BCH],
                        blob[:, C_BDM + kc * 64 : C_BDM + (kc + 1) * 64],
                        start=(kc == 0),
                        stop=(kc == 3),
                    )

                # add-trees on V, per half (rows 0-31 start after slot 3)
                DOTL = accs.tile([BCH, NLOC], f32, name="DOTL", tag="DOTL")
                SQL = accs.tile([BCH, NLOC], f32, name="SQL", tag="SQL")
                HR = NLOC // 2
                for hh, (src_, dst) in enumerate(
                    ((P, DOTL), (Q, SQL), (P, DOTL), (Q, SQL))
                ):
                    half = hh // 2
                    r0 = half * HR
                    nm = ("d", "s")[hh % 2] + str(half)
                    cur = src_[:, r0 * 128 : (r0 + HR) * 128]
                    w = 128
                    while w > 4:
                        w //= 2
                        nxt = tree.tile(
                            [BCH, HR * w], bf16, name=f"t{nm}{w}", tag=f"t{nm}{w}"
                        )
                        nc.vector.tensor_add(
                            nxt.rearrange("p (r k) -> p r k", k=w),
                            cur.rearrange("p (r k) -> p r k", k=2 * w)[:, :, :w],
                            cur.rearrange("p (r k) -> p r k", k=2 * w)[:, :, w:],
                        )
                        cur = nxt
                    t2 = tree.tile([BCH, HR * 2], f32, name=f"t2{nm}", tag=f"t2{nm}")
                    nc.vector.tensor_add(
                        t2.rearrange("p (r k) -> p r k", k=2),
                        cur.rearrange("p (r k) -> p r k", k=4)[:, :, :2],
                        cur.rearrange("p (r k) -> p r k", k=4)[:, :, 2:],
                    )
                    nc.vector.tensor_add(
                        dst[:, r0 : r0 + HR],
                        t2.rearrange("p (r k) -> p r k", k=2)[:, :, 0],
                        t2.rearrange("p (r k) -> p r k", k=2)[:, :, 1],
                    )
                # diag elementwise for this bc (fills V/S gaps)
                xvd = blob[:, C_XVD + bc * 512 : C_XVD + (bc + 1) * 512]
                xle = blob[:, C_XLE + bc * 512 : C_XLE + (bc + 1) * 512]
                expd = tree.tile([BCH, NLOC * FIN], bf16, name="expd", tag="expd")
                nc.scalar.activation(out=expd, in_=pdg, func=AF.Exp)
                sq2 = tree.tile([BCH, NLOC * FIN], bf16, name="sq2", tag="sq2")
                nc.scalar.activation(out=sq2, in_=pdg, func=AF.Exp, scale=2.0)
                SQD = accs.tile([BCH, NLOC], f32, name="SQD", tag="SQD")
                nc.vector.tensor_reduce(
                    out=SQD,
                    in_=sq2.rearrange("p (r f) -> p r f", f=FIN),
                    axis=mybir.AxisListType.X,
                    op=ALU.add,
                )
                prd = tree.tile([BCH, NLOC * FIN], bf16, name="prd", tag="prd")
                nc.vector.tensor_mul(prd, expd, xvd)
                DOTD = accs.tile([BCH, NLOC], f32, name="DOTD", tag="DOTD")
                nc.vector.tensor_reduce(
                    out=DOTD,
                    in_=prd.rearrange("p (r f) -> p r f", f=FIN),
                    axis=mybir.AxisListType.X,
                    op=ALU.add,
                )
                prl = tree.tile([BCH, NLOC * FIN], bf16, name="prl", tag="prl")
                nc.vector.tensor_mul(prl, expd, xle)
                LDS = accs.tile([BCH, NLOC], f32, name="LDS", tag="LDS")
                nc.vector.tensor_reduce(
                    out=LDS,
                    in_=prl.rearrange("p (r f) -> p r f", f=FIN),
                    axis=mybir.AxisListType.X,
                    op=ALU.add,
                )
                st[bc] = dict(
                    pex=pex, pdg=pdg, DOTL=DOTL, SQL=SQL,
                    SQD=SQD, DOTD=DOTD, LDS=LDS,
                )

            # ---- phase C: assembly ----
            for bc in range(2):
                s_ = st[bc]
                sqn = accs.tile([BCH, NLOC], f32, name="sqn", tag="sqn")
                nc.vector.tensor_add(sqn, s_["SQL"], s_["SQD"])
                nc.vector.tensor_add(sqn, sqn, s_["pex"][:, 2 * NLOC : 3 * NLOC])
                dot = accs.tile([BCH, NLOC], f32, name="dot", tag="dot")
                nc.vector.tensor_add(dot, s_["DOTL"], s_["DOTD"])
                nc.vector.tensor_add(dot, dot, s_["pex"][:, 3 * NLOC :])
                s_.update(sqn=sqn, dot=dot)
            for bc in range(2):
                s_ = st[bc]
                l1 = accs.tile([BCH, NLOC], f32, name="l1", tag="l1")
                nc.scalar.activation(out=l1, in_=s_["sqn"], func=AF.Ln)
                l2 = accs.tile([BCH, NLOC], f32, name="l2", tag="l2")
                nc.scalar.activation(out=l2, in_=s_["LDS"], func=AF.Ln)
                s_.update(l1=l1, l2=l2)
            for bc in range(2):
                s_ = st[bc]
                m1 = accs.tile([BCH, NLOC], f32, name="m1", tag="m1")
                nc.scalar.mul(m1, s_["l1"], -0.5)
                u = accs.tile([BCH, NLOC], f32, name="u", tag="u")
                nc.vector.tensor_add(u, s_["pex"][:, :NLOC], m1)
                s_.update(u=u)
            for bc in range(2):
                s_ = st[bc]
                sc = accs.tile([BCH, NLOC], f32, name="sc", tag="sc")
                nc.scalar.activation(out=sc, in_=s_["u"], func=AF.Exp)
                s_.update(sc=sc)
            for bc in range(2):
                b0 = bc * BCH
                s_ = st[bc]
                yv = accs.tile([BCH, NLOC], f32, name="yv", tag="yv")
                nc.vector.tensor_mul(yv, s_["dot"], s_["sc"])
                yb = accs.tile([BCH, NLOC], f32, name="yb", tag="yb")
                nc.vector.tensor_add(yb, yv, s_["pex"][:, NLOC : 2 * NLOC])
                ld = accs.tile([BCH, NLOC], f32, name="ld", tag="ld")
                nc.vector.tensor_add(ld, s_["u"], s_["l2"])
                ob = accs.tile([BCH, NLOC, 2], f32, name="ob", tag="ob")
                nc.gpsimd.tensor_copy(out=ob[:, :, 0], in_=yb)
                nc.gpsimd.tensor_copy(out=ob[:, :, 1], in_=ld)
                nc.sync.dma_start(out=d_out[b0 : b0 + BCH, :, :], in_=ob)

    nc.compile()
    return nc


def _host_prep(x, c, Wd, bd, Wa, ba, Wb, bb):
    """Build the 8 per-core input maps."""
    import ml_dtypes

    bf = ml_dtypes.bfloat16
    x = np.ascontiguousarray(x, dtype=np.float32)
    c = np.ascontiguousarray(c, dtype=np.float32)
    Wd5 = np.ascontiguousarray(Wd, dtype=np.float32).reshape(CDIM, NCH, FOUT, NCH, FIN)
    bd4 = np.ascontiguousarray(bd, dtype=np.float32).reshape(NCH, FOUT, NCH, FIN)
    Wa_ = np.ascontiguousarray(Wa, dtype=np.float32)
    Wb_ = np.ascontiguousarray(Wb, dtype=np.float32)
    ba = np.ascontiguousarray(ba, dtype=np.float32)
    bb = np.ascontiguousarray(bb, dtype=np.float32)

    cT = np.ascontiguousarray(c.T)
    xv = np.ascontiguousarray(x[:, :, 0])
    xl = np.ascontiguousarray(x[:, :, 1])
    xvT = np.ascontiguousarray(xv.T)

    in_maps = []
    for k in range(NCORES):
        chs = _channels(k)
        blob = np.zeros((CDIM, NCOL), dtype=np.float32)
        row1 = np.zeros((1, 704), dtype=np.float32)
        wfp = np.zeros((128, 13 * 1024), dtype=np.float32)
        wap = np.zeros((56, 4 * 1024), dtype=np.float32)
        wbp = np.zeros((120, 4 * 1024), dtype=np.float32)
        packs = {"f": wfp, "a": wap, "b": wbp}

        blob[:, C_CT : C_CT + 256] = cT
        for t in range(4):
            blob[:, C_XVT + t * 256 : C_XVT + (t + 1) * 256] = xvT[
                t * 128 : (t + 1) * 128, :
            ]
        for bc in range(2):
            blob[:, C_CB8 + bc * 1024 : C_CB8 + (bc + 1) * 1024] = np.tile(
                c[bc * BCH : (bc + 1) * BCH, :], (1, 8)
            )

        bdm = np.zeros((I, NLOC), dtype=np.float32)
        for j, ch in enumerate(chs):
            w = ch * FIN
            Gall = Wd5[:, ch, :, :, :].reshape(CDIM, FOUT, I)  # [128, 8, 512]
            for t, ext in enumerate(EXT_OF_J[j]):
                i0 = 128 * t
                n = max(0, min(w - i0, ext))
                if n > 0:
                    kind, off = SLAB_PACK[(j, t)]
                    seg = Gall[:, :, i0 : i0 + n]  # [128, 8, n]
                    packs[kind][:n, off : off + 1024] = seg.transpose(2, 1, 0).reshape(
                        n, 8 * 128
                    )
            for fo in range(FOUT):
                r = j * FOUT + fo
                G = Gall[:, fo, :w]  # [128, w]
                if w > 0:
                    if w <= 128:
                        blob[:, C_CSL + r * 128 : C_CSL + r * 128 + w] = G
                    else:
                        M = G.astype(np.float64) @ G.astype(np.float64).T
                        M += np.eye(CDIM) * (1e-12 * np.trace(M) / CDIM)
                        blob[:, C_CSL + r * 128 : C_CSL + (r + 1) * 128] = (
                            np.linalg.cholesky(M)
                        )
                blob[:, C_WDD + r * FIN : C_WDD + (r + 1) * FIN] = Wd5[:, ch, fo, ch, :]
                row1[0, r * FIN : (r + 1) * FIN] = bd4[ch, fo, ch, :]
                bd_low = bd4[ch, fo, :ch, :].reshape(-1)
                bdm[:w, r] = bd_low
                if w > 0:
                    blob[:, C_EW + 128 + r] = 2.0 * (G @ bd_low)
                    row1[0, 512 + 128 + r] = np.dot(bd_low, bd_low)
            rows = slice(ch * FOUT, (ch + 1) * FOUT)
            blob[:, C_EW + j * FOUT : C_EW + (j + 1) * FOUT] = Wa_[:, rows]
            blob[:, C_EW + 64 + j * FOUT : C_EW + 64 + (j + 1) * FOUT] = Wb_[:, rows]
            row1[0, 512 + j * FOUT : 512 + (j + 1) * FOUT] = ba[rows]
            row1[0, 512 + 64 + j * FOUT : 512 + 64 + (j + 1) * FOUT] = bb[rows]

        for kc in range(4):
            blob[:, C_BDM + kc * 64 : C_BDM + (kc + 1) * 64] = bdm[
                kc * 128 : (kc + 1) * 128, :
            ]

        xvd = np.empty((B, NLOC * FIN), dtype=np.float32)
        xled = np.empty((B, NLOC * FIN), dtype=np.float32)
        for r in range(NLOC):
            ch = chs[r // FOUT]
            xvd[:, r * FIN : (r + 1) * FIN] = xv[:, ch * FIN : (ch + 1) * FIN]
            xled[:, r * FIN : (r + 1) * FIN] = xl[:, ch * FIN : (ch + 1) * FIN]
        np.exp(xled, out=xled)
        for bc in range(2):
            blob[:, C_XVD + bc * 512 : C_XVD + (bc + 1) * 512] = xvd[
                bc * BCH : (bc + 1) * BCH, :
            ]
            blob[:, C_XLE + bc * 512 : C_XLE + (bc + 1) * 512] = xled[
                bc * BCH : (bc + 1) * BCH, :
            ]

        in_maps.append(
            {
                "blob": blob.astype(bf),
                "row1": row1.astype(bf),
                "wf": wfp.astype(bf),
                "wa": wap.astype(bf),
                "wb": wbp.astype(bf),
            }
        )
    return in_maps


def kernel(x, c, Wd, bd, Wa, ba, Wb, bb, _trace=False, _tmpdir=None):
    global _NC
    from concourse.bass_utils import run_bass_kernel_spmd

    if _NC is None:
        _NC = _build_nc()
    in_maps = _host_prep(x, c, Wd, bd, Wa, ba, Wb, bb)
    res = run_bass_kernel_spmd(
        _NC, in_maps, core_ids=list(range(NCORES)), trace=_trace, tmpdir=_tmpdir
    )

    out = np.empty((B, O, 2), dtype=np.float32)
    for k in range(NCORES):
        ok = res.results[k]["out"]
        for j, ch in enumerate(_channels(k)):
            out[:, ch * FOUT : (ch + 1) * FOUT, :] = ok[:, j * FOUT : (j + 1) * FOUT, :]
    if _trace:
        return out, res
    return out


# revision 15
# speedup vs baseline: 1.4206x; 1.0016x over previous
"""Trainium2 Bass kernel for nn_CLinear_6768868459230.

Context-conditioned block-autoregressive linear layer (MAF-style):
  wdir = c @ Wd + bd                      [B, O, I]
  w    = exp(wdir)*mask_diag + wdir*mask_lower
  sqn  = sum(w^2, axis=I)
  y    = (w / sqrt(sqn) * exp(wamp)) @ xv + bias
  logdet = logsumexp over diag block of (wdir - 0.5 log sqn + wamp + xl)

Sharding: tensor-parallel over the O=512 output rows; core k owns channels
{k, 15-k, 16+k, 31-k, ...} so the triangular work is the same on every core.

Algorithm (v3): never materialize t = c @ Wd_lower (whose [B, ~17k]
elementwise reductions bound the baseline).  Instead per output row o:
  dot_lower = c^T (W_o x)   -> R[b,(o,k)] = sum_i W[k,o,i] x[b,i] on
     TensorE (i-chunked, true-K, no padding cost), then one multiply by c
     and a bf16 add-tree: 128 terms/row instead of up to 504.
  sqn_lower = |C_o^T c|^2 + 2(G_o bd)^T c + |bd|^2, C_o = G_o (w<=128) or
     chol(G_o G_o^T): ScalarE squares the 128-wide s = C^T c straight out
     of PSUM (bf16 out), GpSimd segment-reduces.
Engine split per 128-sample chunk: TensorE ~30k cols; PSUM evacuation
split S (squares + 4 slot copies) / V (4 direct-PSUM mults); V runs the
bf16 dot-tree; GpSimd the square-reduce, diag products and output
interleave.  All operands bf16; DMAs consolidated into ~14 transfers.
"""

import numpy as np

NCH, FIN, FOUT, CDIM, B = 64, 8, 8, 128, 256
I = NCH * FIN
O = NCH * FOUT
NCORES = 8
NLOC = 64  # output rows per core
BCH = 128  # batch chunk (SBUF partitions)

# per-slot i-chunk extents (max over cores of w=8*ch for that slot)
EXT_OF_J = [
    [56],
    [120],
    [128, 56],
    [128, 120],
    [128, 128, 56],
    [128, 128, 120],
    [128, 128, 128, 56],
    [128, 128, 128, 120],
]
# slab packing: full 128-row slabs in one [128, 13*1024] tensor; the
# 56-row and 120-row tails in [56, 4*1024] and [120, 4*1024] tensors.
SLAB_PACK = {}
_nf = _n56 = _n120 = 0
for _j in range(8):
    for _t, _e in enumerate(EXT_OF_J[_j]):
        if _e == 128:
            SLAB_PACK[(_j, _t)] = ("f", _nf * 1024)
            _nf += 1
        elif _e == 56:
            SLAB_PACK[(_j, _t)] = ("a", _n56 * 1024)
            _n56 += 1
        else:
            SLAB_PACK[(_j, _t)] = ("b", _n120 * 1024)
            _n120 += 1

# col offsets inside the consolidated [128, NCOL] blob
C_CT = 0          # [128, 256]   c transposed
C_XVT = 256       # [128, 4*256] xv transposed, 4 i-chunks
C_CB8 = 1280      # [128, 2*1024] c tiled 8x per batch chunk
C_CSL = 3328      # [128, 64*128] chol/G factors
C_WDD = 11520     # [128, 512]   diag weights
C_BDM = 12032     # [128, 4*64]  bd-lower matvec weights, 4 i-chunks
C_XVD = 12288     # [128, 2*512] diag-gathered xv per batch chunk
C_XLE = 13312     # [128, 2*512] diag-gathered exp(xl)
C_EW = 14336      # [128, 192]   wamp | bias | 2cg weights
NCOL = 14528


def _channels(k):
    return [k, 15 - k, 16 + k, 31 - k, 32 + k, 47 - k, 48 + k, 63 - k]


_NC = None


def _build_nc():
    import concourse.bacc as bacc
    import concourse.tile as tile
    from concourse import mybir

    f32 = mybir.dt.float32
    bf16 = mybir.dt.bfloat16
    AF = mybir.ActivationFunctionType
    ALU = mybir.AluOpType

    nc = bacc.Bacc(None, target_bir_lowering=False)

    d_blob = nc.dram_tensor("blob", [CDIM, NCOL], bf16, kind="ExternalInput")
    d_row1 = nc.dram_tensor("row1", [1, 704], bf16, kind="ExternalInput")
    d_wf = nc.dram_tensor("wf", [128, 13 * 1024], bf16, kind="ExternalInput")
    d_wa = nc.dram_tensor("wa", [56, 4 * 1024], bf16, kind="ExternalInput")
    d_wb = nc.dram_tensor("wb", [120, 4 * 1024], bf16, kind="ExternalInput")
    d_out = nc.dram_tensor("out", [B, NLOC, 2], f32, kind="ExternalOutput")

    with tile.TileContext(nc) as tc:
        with (
            tc.tile_pool(name="consts", bufs=1) as consts,
            tc.tile_pool(name="rsb", bufs=3) as rsb,
            tc.tile_pool(name="big", bufs=2) as big,
            tc.tile_pool(name="tree", bufs=1) as tree,
            tc.tile_pool(name="accs", bufs=2) as accs,
            tc.tile_pool(name="rp", bufs=2, space="PSUM") as rp,
            tc.tile_pool(name="sp", bufs=1, space="PSUM") as sp,
            tc.tile_pool(name="miscp", bufs=1, space="PSUM") as miscp,
            tc.tile_pool(name="extp", bufs=1, space="PSUM") as extp,
        ):
            blob = consts.tile([CDIM, NCOL], bf16)
            row1 = consts.tile([1, 704], bf16)
            wf = consts.tile([128, 13 * 1024], bf16)
            wa = consts.tile([56, 4 * 1024], bf16)
            wb = consts.tile([120, 4 * 1024], bf16)
            wtile = {"f": wf, "a": wa, "b": wb}
            ones_sb = consts.tile([1, BCH], bf16)
            nc.vector.memset(ones_sb, 1.0)

            # Stream inputs in compute-use order: small operand blocks first
            # so matmuls start within ~2us, then csl/W slabs slot by slot.
            # sync + scalar queues carry the bulk (HW DGEs); gpsimd carries
            # the small early blocks (its queue is otherwise idle).
            nc.gpsimd.dma_start(out=blob[:, :C_CSL], in_=d_blob[:, :C_CSL])
            nc.gpsimd.dma_start(out=blob[:, C_WDD:], in_=d_blob[:, C_WDD:])
            nc.gpsimd.dma_start(out=row1, in_=d_row1[:, :])
            # per-slot weight arrivals, alternating sync/scalar
            _sq = [nc.sync, nc.scalar]
            _qi = 0

            def _q():
                nonlocal _qi
                q = _sq[_qi % 2]
                _qi += 1
                return q

            for j in range(8):
                _q().dma_start(
                    out=blob[:, C_CSL + j * 1024 : C_CSL + (j + 1) * 1024],
                    in_=d_blob[:, C_CSL + j * 1024 : C_CSL + (j + 1) * 1024],
                )
                for t in range(len(EXT_OF_J[j])):
                    kind, off = SLAB_PACK[(j, t)]
                    wt = wtile[kind]
                    dt = {"f": d_wf, "a": d_wa, "b": d_wb}[kind]
                    _q().dma_start(
                        out=wt[:, off : off + 1024], in_=dt[:, off : off + 1024]
                    )

            st = {}
            stA = {}
            # ---- phase A: extras first, then slot-interleaved over both
            # batch chunks so each weight slab is consumed as it arrives ----
            pex2 = extp.tile([BCH, 2, 4 * NLOC], f32, name="pex2", tag="pex2")
            for bc in range(2):
                b0 = bc * BCH
                ctl = blob[:, C_CT + b0 : C_CT + b0 + BCH]
                pex = pex2[:, bc, :]
                nc.tensor.matmul(
                    pex[:, : 3 * NLOC],
                    ctl,
                    blob[:, C_EW : C_EW + 192],
                    start=True,
                    stop=False,
                )
                nc.tensor.matmul(
                    pex[:, : 3 * NLOC],
                    ones_sb,
                    row1[:, 512:704],
                    start=False,
                    stop=True,
                )
                P = big.tile([BCH, NLOC * 128], bf16, name="P", tag="P")
                Q = big.tile([BCH, NLOC * 128], bf16, name="Q", tag="Q")
                stA[bc] = (pex, P, Q)

            for j in range(8):
                c0 = j * 1024
                for bc in range(2):
                    b0 = bc * BCH
                    ctl = blob[:, C_CT + b0 : C_CT + b0 + BCH]
                    cb8 = blob[:, C_CB8 + bc * 1024 : C_CB8 + (bc + 1) * 1024]
                    pex, P, Q = stA[bc]
                    # chol matmul first (S consumes while R matmuls run)
                    spj = sp.tile([BCH, 1024], f32, name="spj", tag="spj")
                    for h in range(2):
                        nc.tensor.matmul(
                            spj[:, h * 512 : (h + 1) * 512],
                            ctl,
                            blob[
                                :, C_CSL + c0 + h * 512 : C_CSL + c0 + (h + 1) * 512
                            ],
                            start=True,
                            stop=True,
                        )
                    rpj = rp.tile([BCH, 1024], f32, name="rpj", tag="rpj")
                    nch = len(EXT_OF_J[j])
                    for t, ext in enumerate(EXT_OF_J[j]):
                        kind, off = SLAB_PACK[(j, t)]
                        wt = wtile[kind]
                        xcol = C_XVT + t * 256 + b0
                        for h in range(2):
                            nc.tensor.matmul(
                                rpj[:, h * 512 : (h + 1) * 512],
                                blob[:ext, xcol : xcol + BCH],
                                wt[:ext, off + h * 512 : off + (h + 1) * 512],
                                start=(t == 0),
                                stop=(t == nch - 1),
                            )
                    # S: square the chol output straight out of PSUM
                    nc.scalar.activation(
                        out=Q[:, c0 : c0 + 1024], in_=spj, func=AF.Square
                    )
                    if j < 6:
                        # S copies R to SBUF; V multiplies at 2x
                        rsj = rsb.tile([BCH, 1024], bf16, name="rsj", tag="rsj")
                        nc.scalar.activation(out=rsj, in_=rpj, func=AF.Copy)
                        nc.vector.tensor_mul(P[:, c0 : c0 + 1024], rsj, cb8)
                    else:
                        # V multiplies straight out of PSUM
                        nc.vector.tensor_mul(P[:, c0 : c0 + 1024], rpj, cb8)

            for bc in range(2):
                b0 = bc * BCH
                ctl = blob[:, C_CT + b0 : C_CT + b0 + BCH]
                pex, P, Q = stA[bc]
                # dotbd into pex[:, 192:256]
                for kc in range(4):
                    xcol = C_XVT + kc * 256 + b0
                    nc.tensor.matmul(
                        pex[:, 3 * NLOC :],
                        blob[:, xcol : xcol + BCH],
                        blob[:, C_BDM + kc * 64 : C_BDM + (kc + 1) * 64],
                        start=(kc == 0),
                        stop=(kc == 3),
                    )
                # add-trees on V, per half (rows 0-31 start after slot 3)
                DOTL = accs.tile([BCH, NLOC], f32, name="DOTL", tag="DOTL")
                SQL = accs.tile([BCH, NLOC], f32, name="SQL", tag="SQL")
                HR = NLOC // 2
                for hh, (src_, dst) in enumerate(
                    ((P, DOTL), (Q, SQL), (P, DOTL), (Q, SQL))
                ):
                    half = hh // 2
                    r0 = half * HR
                    nm = ("d", "s")[hh % 2] + str(half)
                    cur = src_[:, r0 * 128 : (r0 + HR) * 128]
                    w = 128
                    while w > 4:
                        w //= 2
                        nxt = tree.tile(
                            [BCH, HR * w], bf16, name=f"t{nm}{w}", tag=f"t{nm}{w}"
                        )
                        nc.vector.tensor_add(
                            nxt.rearrange("p (r k) -> p r k", k=w),
                            cur.rearrange("p (r k) -> p r k", k=2 * w)[:, :, :w],
                            cur.rearrange("p (r k) -> p r k", k=2 * w)[:, :, w:],
                        )
                        cur = nxt
                    t2 = tree.tile([BCH, HR * 2], f32, name=f"t2{nm}", tag=f"t2{nm}")
                    nc.vector.tensor_add(
                        t2.rearrange("p (r k) -> p r k", k=2),
                        cur.rearrange("p (r k) -> p r k", k=4)[:, :, :2],
                        cur.rearrange("p (r k) -> p r k", k=4)[:, :, 2:],
                    )
                    nc.vector.tensor_add(
                        dst[:, r0 : r0 + HR],
                        t2.rearrange("p (r k) -> p r k", k=2)[:, :, 0],
                        t2.rearrange("p (r k) -> p r k", k=2)[:, :, 1],
                    )
                # diag block (matmul late: PSUM bank freed by slot loop)
                pdg = miscp.tile([BCH, NLOC * FIN], f32, name="pdg", tag="pdg")
                nc.tensor.matmul(
                    pdg, ctl, blob[:, C_WDD : C_WDD + 512], start=True, stop=False
                )
                nc.tensor.matmul(pdg, ones_sb, row1[:, :512], start=False, stop=True)
                xvd = blob[:, C_XVD + bc * 512 : C_XVD + (bc + 1) * 512]
                xle = blob[:, C_XLE + bc * 512 : C_XLE + (bc + 1) * 512]
                expd = tree.tile([BCH, NLOC * FIN], bf16, name="expd", tag="expd")
                nc.scalar.activation(out=expd, in_=pdg, func=AF.Exp)
                sq2 = tree.tile([BCH, NLOC * FIN], bf16, name="sq2", tag="sq2")
                nc.scalar.activation(out=sq2, in_=pdg, func=AF.Exp, scale=2.0)
                SQD = accs.tile([BCH, NLOC], f32, name="SQD", tag="SQD")
                nc.vector.tensor_reduce(
                    out=SQD,
                    in_=sq2.rearrange("p (r f) -> p r f", f=FIN),
                    axis=mybir.AxisListType.X,
                    op=ALU.add,
                )
                prd = tree.tile([BCH, NLOC * FIN], bf16, name="prd", tag="prd")
                nc.vector.tensor_mul(prd, expd, xvd)
                DOTD = accs.tile([BCH, NLOC], f32, name="DOTD", tag="DOTD")
                nc.vector.tensor_reduce(
                    out=DOTD,
                    in_=prd.rearrange("p (r f) -> p r f", f=FIN),
                    axis=mybir.AxisListType.X,
                    op=ALU.add,
                )
                prl = tree.tile([BCH, NLOC * FIN], bf16, name="prl", tag="prl")
                nc.vector.tensor_mul(prl, expd, xle)
                LDS = accs.tile([BCH, NLOC], f32, name="LDS", tag="LDS")
                nc.vector.tensor_reduce(
                    out=LDS,
                    in_=prl.rearrange("p (r f) -> p r f", f=FIN),
                    axis=mybir.AxisListType.X,
                    op=ALU.add,
                )
                st[bc] = dict(
                    pex=pex, pdg=pdg, DOTL=DOTL, SQL=SQL,
                    SQD=SQD, DOTD=DOTD, LDS=LDS,
                )

            # ---- phase C: assembly ----
            for bc in range(2):
                s_ = st[bc]
                sqn = accs.tile([BCH, NLOC], f32, name="sqn", tag="sqn")
                nc.vector.tensor_add(sqn, s_["SQL"], s_["SQD"])
                nc.vector.tensor_add(sqn, sqn, s_["pex"][:, 2 * NLOC : 3 * NLOC])
                dot = accs.tile([BCH, NLOC], f32, name="dot", tag="dot")
                nc.vector.tensor_add(dot, s_["DOTL"], s_["DOTD"])
                nc.vector.tensor_add(dot, dot, s_["pex"][:, 3 * NLOC :])
                s_.update(sqn=sqn, dot=dot)
            for bc in range(2):
                s_ = st[bc]
                l1 = accs.tile([BCH, NLOC], f32, name="l1", tag="l1")
                nc.scalar.activation(out=l1, in_=s_["sqn"], func=AF.Ln)
                l2 = accs.tile([BCH, NLOC], f32, name="l2", tag="l2")
                nc.scalar.activation(out=l2, in_=s_["LDS"], func=AF.Ln)
                s_.update(l1=l1, l2=l2)
            for bc in range(2):
                s_ = st[bc]
                m1 = accs.tile([BCH, NLOC], f32, name="m1", tag="m1")
                nc.scalar.mul(m1, s_["l1"], -0.5)
                u = accs.tile([BCH, NLOC], f32, name="u", tag="u")
                nc.vector.tensor_add(u, s_["pex"][:, :NLOC], m1)
                s_.update(u=u)
            for bc in range(2):
                s_ = st[bc]
                sc = accs.tile([BCH, NLOC], f32, name="sc", tag="sc")
                nc.scalar.activation(out=sc, in_=s_["u"], func=AF.Exp)
                s_.update(sc=sc)
            for bc in range(2):
                b0 = bc * BCH
                s_ = st[bc]
                yv = accs.tile([BCH, NLOC], f32, name="yv", tag="yv")
                nc.vector.tensor_mul(yv, s_["dot"], s_["sc"])
                yb = accs.tile([BCH, NLOC], f32, name="yb", tag="yb")
                nc.vector.tensor_add(yb, yv, s_["pex"][:, NLOC : 2 * NLOC])
                ld = accs.tile([BCH, NLOC], f32, name="ld", tag="ld")
                nc.vector.tensor_add(ld, s_["u"], s_["l2"])
                ob = accs.tile([BCH, NLOC, 2], f32, name="ob", tag="ob")
                nc.gpsimd.tensor_copy(out=ob[:, :, 0], in_=yb)
                nc.gpsimd.tensor_copy(out=ob[:, :, 1], in_=ld)
                nc.sync.dma_start(out=d_out[b0 : b0 + BCH, :, :], in_=ob)

    nc.compile()
    return nc


def _host_prep(x, c, Wd, bd, Wa, ba, Wb, bb):
    """Build the 8 per-core input maps."""
    import ml_dtypes

    bf = ml_dtypes.bfloat16
    x = np.ascontiguousarray(x, dtype=np.float32)
    c = np.ascontiguousarray(c, dtype=np.float32)
    Wd5 = np.ascontiguousarray(Wd, dtype=np.float32).reshape(CDIM, NCH, FOUT, NCH, FIN)
    bd4 = np.ascontiguousarray(bd, dtype=np.float32).reshape(NCH, FOUT, NCH, FIN)
    Wa_ = np.ascontiguousarray(Wa, dtype=np.float32)
    Wb_ = np.ascontiguousarray(Wb, dtype=np.float32)
    ba = np.ascontiguousarray(ba, dtype=np.float32)
    bb = np.ascontiguousarray(bb, dtype=np.float32)

    cT = np.ascontiguousarray(c.T)
    xv = np.ascontiguousarray(x[:, :, 0])
    xl = np.ascontiguousarray(x[:, :, 1])
    xvT = np.ascontiguousarray(xv.T)

    in_maps = []
    for k in range(NCORES):
        chs = _channels(k)
        blob = np.zeros((CDIM, NCOL), dtype=np.float32)
        row1 = np.zeros((1, 704), dtype=np.float32)
        wfp = np.zeros((128, 13 * 1024), dtype=np.float32)
        wap = np.zeros((56, 4 * 1024), dtype=np.float32)
        wbp = np.zeros((120, 4 * 1024), dtype=np.float32)
        packs = {"f": wfp, "a": wap, "b": wbp}

        blob[:, C_CT : C_CT + 256] = cT
        for t in range(4):
            blob[:, C_XVT + t * 256 : C_XVT + (t + 1) * 256] = xvT[
                t * 128 : (t + 1) * 128, :
            ]
        for bc in range(2):
            blob[:, C_CB8 + bc * 1024 : C_CB8 + (bc + 1) * 1024] = np.tile(
                c[bc * BCH : (bc + 1) * BCH, :], (1, 8)
            )

        bdm = np.zeros((I, NLOC), dtype=np.float32)
        for j, ch in enumerate(chs):
            w = ch * FIN
            Gall = Wd5[:, ch, :, :, :].reshape(CDIM, FOUT, I)  # [128, 8, 512]
            for t, ext in enumerate(EXT_OF_J[j]):
                i0 = 128 * t
                n = max(0, min(w - i0, ext))
                if n > 0:
                    kind, off = SLAB_PACK[(j, t)]
                    seg = Gall[:, :, i0 : i0 + n]  # [128, 8, n]
                    packs[kind][:n, off : off + 1024] = seg.transpose(2, 1, 0).reshape(
                        n, 8 * 128
                    )
            for fo in range(FOUT):
                r = j * FOUT + fo
                G = Gall[:, fo, :w]  # [128, w]
                if w > 0:
                    if w <= 128:
                        blob[:, C_CSL + r * 128 : C_CSL + r * 128 + w] = G
                    else:
                        M = G.astype(np.float64) @ G.astype(np.float64).T
                        M += np.eye(CDIM) * (1e-12 * np.trace(M) / CDIM)
                        blob[:, C_CSL + r * 128 : C_CSL + (r + 1) * 128] = (
                            np.linalg.cholesky(M)
                        )
                blob[:, C_WDD + r * FIN : C_WDD + (r + 1) * FIN] = Wd5[:, ch, fo, ch, :]
                row1[0, r * FIN : (r + 1) * FIN] = bd4[ch, fo, ch, :]
                bd_low = bd4[ch, fo, :ch, :].reshape(-1)
                bdm[:w, r] = bd_low
                if w > 0:
                    blob[:, C_EW + 128 + r] = 2.0 * (G @ bd_low)
                    row1[0, 512 + 128 + r] = np.dot(bd_low, bd_low)
            rows = slice(ch * FOUT, (ch + 1) * FOUT)
            blob[:, C_EW + j * FOUT : C_EW + (j + 1) * FOUT] = Wa_[:, rows]
            blob[:, C_EW + 64 + j * FOUT : C_EW + 64 + (j + 1) * FOUT] = Wb_[:, rows]
            row1[0, 512 + j * FOUT : 512 + (j + 1) * FOUT] = ba[rows]
            row1[0, 512 + 64 + j * FOUT : 512 + 64 + (j + 1) * FOUT] = bb[rows]

        for kc in range(4):
            blob[:, C_BDM + kc * 64 : C_BDM + (kc + 1) * 64] = bdm[
                kc * 128 : (kc + 1) * 128, :
            ]

        xvd = np.empty((B, NLOC * FIN), dtype=np.float32)
        xled = np.empty((B, NLOC * FIN), dtype=np.float32)
        for r in range(NLOC):
            ch = chs[r // FOUT]
            xvd[:, r * FIN : (r + 1) * FIN] = xv[:, ch * FIN : (ch + 1) * FIN]
            xled[:, r * FIN : (r + 1) * FIN] = xl[:, ch * FIN : (ch + 1) * FIN]
        np.exp(xled, out=xled)
        for bc in range(2):
            blob[:, C_XVD + bc * 512 : C_XVD + (bc + 1) * 512] = xvd[
                bc * BCH : (bc + 1) * BCH, :
            ]
            blob[:, C_XLE + bc * 512 : C_XLE + (bc + 1) * 512] = xled[
                bc * BCH : (bc + 1) * BCH, :
            ]

        in_maps.append(
            {
                "blob": blob.astype(bf),
                "row1": row1.astype(bf),
                "wf": wfp.astype(bf),
                "wa": wap.astype(bf),
                "wb": wbp.astype(bf),
            }
        )
    return in_maps


def kernel(x, c, Wd, bd, Wa, ba, Wb, bb, _trace=False, _tmpdir=None):
    global _NC
    from concourse.bass_utils import run_bass_kernel_spmd

    if _NC is None:
        _NC = _build_nc()
    in_maps = _host_prep(x, c, Wd, bd, Wa, ba, Wb, bb)
    res = run_bass_kernel_spmd(
        _NC, in_maps, core_ids=list(range(NCORES)), trace=_trace, tmpdir=_tmpdir
    )

    out = np.empty((B, O, 2), dtype=np.float32)
    for k in range(NCORES):
        ok = res.results[k]["out"]
        for j, ch in enumerate(_channels(k)):
            out[:, ch * FOUT : (ch + 1) * FOUT, :] = ok[:, j * FOUT : (j + 1) * FOUT, :]
    if _trace:
        return out, res
    return out


# revision 16
# speedup vs baseline: 1.5310x; 1.0777x over previous
"""Trainium2 Bass kernel for nn_CLinear_6768868459230.

Context-conditioned block-autoregressive linear layer (MAF-style):
  wdir = c @ Wd + bd                      [B, O, I]
  w    = exp(wdir)*mask_diag + wdir*mask_lower
  sqn  = sum(w^2, axis=I)
  y    = (w / sqrt(sqn) * exp(wamp)) @ xv + bias
  logdet = logsumexp over diag block of (wdir - 0.5 log sqn + wamp + xl)

Sharding: tensor-parallel over the O=512 output rows; core k owns channels
{k, 15-k, 16+k, 31-k, ...} so the triangular work is the same on every core.

Algorithm (v3): never materialize t = c @ Wd_lower (whose [B, ~17k]
elementwise reductions bound the baseline).  Instead per output row o:
  dot_lower = c^T (W_o x)   -> R[b,(o,k)] = sum_i W[k,o,i] x[b,i] on
     TensorE (i-chunked, true-K, no padding cost), then one multiply by c
     and a bf16 add-tree: 128 terms/row instead of up to 504.
  sqn_lower = |C_o^T c|^2 + 2(G_o bd)^T c + |bd|^2, C_o = G_o (w<=128) or
     chol(G_o G_o^T): ScalarE squares the 128-wide s = C^T c straight out
     of PSUM (bf16 out), GpSimd segment-reduces.
Engine split per 128-sample chunk: TensorE ~30k cols; PSUM evacuation
split S (squares + 4 slot copies) / V (4 direct-PSUM mults); V runs the
bf16 dot-tree; GpSimd the square-reduce, diag products and output
interleave.  All operands bf16; DMAs consolidated into ~14 transfers.
"""

import numpy as np

NCH, FIN, FOUT, CDIM, B = 64, 8, 8, 128, 256
I = NCH * FIN
O = NCH * FOUT
NCORES = 8
NLOC = 64  # output rows per core
BCH = 128  # batch chunk (SBUF partitions)

# per-slot i-chunk extents (max over cores of w=8*ch for that slot)
EXT_OF_J = [
    [56],
    [120],
    [128, 56],
    [128, 120],
    [128, 128, 56],
    [128, 128, 120],
    [128, 128, 128, 56],
    [128, 128, 128, 120],
]
# slab packing: full 128-row slabs in one [128, 13*1024] tensor; the
# 56-row and 120-row tails in [56, 4*1024] and [120, 4*1024] tensors.
SLAB_PACK = {}
_nf = _n56 = _n120 = 0
for _j in range(8):
    for _t, _e in enumerate(EXT_OF_J[_j]):
        if _e == 128:
            SLAB_PACK[(_j, _t)] = ("f", _nf * 1024)
            _nf += 1
        elif _e == 56:
            SLAB_PACK[(_j, _t)] = ("a", _n56 * 1024)
            _n56 += 1
        else:
            SLAB_PACK[(_j, _t)] = ("b", _n120 * 1024)
            _n120 += 1

# col offsets inside the consolidated [128, NCOL] blob
C_CT = 0          # [128, 256]   c transposed
C_XVT = 256       # [128, 4*256] xv transposed, 4 i-chunks
C_CB8 = 1280      # [128, 2*1024] c tiled 8x per batch chunk
C_CSL = 3328      # [128, 64*128] chol/G factors
C_WDD = 11520     # [128, 512]   diag weights
C_BDM = 12032     # [128, 4*64]  bd-lower matvec weights, 4 i-chunks
C_XVD = 12288     # [128, 2*512] diag-gathered xv per batch chunk
C_XLE = 13312     # [128, 2*512] diag-gathered exp(xl)
C_EW = 14336      # [128, 192]   wamp | bias | 2cg weights
NCOL = 14528


def _channels(k):
    return [k, 15 - k, 16 + k, 31 - k, 32 + k, 47 - k, 48 + k, 63 - k]


_NC = None


def _build_nc():
    import concourse.bacc as bacc
    import concourse.tile as tile
    from concourse import mybir

    f32 = mybir.dt.float32
    bf16 = mybir.dt.bfloat16
    AF = mybir.ActivationFunctionType
    ALU = mybir.AluOpType

    nc = bacc.Bacc(None, target_bir_lowering=False)

    d_blob = nc.dram_tensor("blob", [CDIM, NCOL], bf16, kind="ExternalInput")
    d_row1 = nc.dram_tensor("row1", [1, 704], bf16, kind="ExternalInput")
    d_wf = nc.dram_tensor("wf", [128, 13 * 1024], bf16, kind="ExternalInput")
    d_wa = nc.dram_tensor("wa", [56, 4 * 1024], bf16, kind="ExternalInput")
    d_wb = nc.dram_tensor("wb", [120, 4 * 1024], bf16, kind="ExternalInput")
    d_out = nc.dram_tensor("out", [B, NLOC, 2], f32, kind="ExternalOutput")

    with tile.TileContext(nc) as tc:
        with (
            tc.tile_pool(name="consts", bufs=1) as consts,
            tc.tile_pool(name="rsb", bufs=3) as rsb,
            tc.tile_pool(name="big", bufs=2) as big,
            tc.tile_pool(name="tree", bufs=1) as tree,
            tc.tile_pool(name="accs", bufs=2) as accs,
            tc.tile_pool(name="rp", bufs=2, space="PSUM") as rp,
            tc.tile_pool(name="sp", bufs=1, space="PSUM") as sp,
            tc.tile_pool(name="miscp", bufs=1, space="PSUM") as miscp,
            tc.tile_pool(name="extp", bufs=1, space="PSUM") as extp,
        ):
            blob = consts.tile([CDIM, NCOL], bf16)
            row1 = consts.tile([1, 704], bf16)
            wf = consts.tile([128, 13 * 1024], bf16)
            wa = consts.tile([56, 4 * 1024], bf16)
            wb = consts.tile([120, 4 * 1024], bf16)
            wtile = {"f": wf, "a": wa, "b": wb}
            ones_sb = consts.tile([1, BCH], bf16)
            nc.vector.memset(ones_sb, 1.0)

            # Stream inputs in compute-use order: small operand blocks first
            # so matmuls start within ~2us, then csl/W slabs slot by slot.
            # sync + scalar queues carry the bulk (HW DGEs); gpsimd carries
            # the small early blocks (its queue is otherwise idle).
            # sync HW queue: lhs operands first (2.8us), then W slabs in
            # slot order; scalar HW queue: chol factors then the small tail;
            # gpsimd SW queue: only the tiny row1.
            nc.sync.dma_start(out=blob[:, :C_CSL], in_=d_blob[:, :C_CSL])
            for j in range(8):
                nc.scalar.dma_start(
                    out=blob[:, C_CSL + j * 1024 : C_CSL + (j + 1) * 1024],
                    in_=d_blob[:, C_CSL + j * 1024 : C_CSL + (j + 1) * 1024],
                )
                for t in range(len(EXT_OF_J[j])):
                    kind, off = SLAB_PACK[(j, t)]
                    wt = wtile[kind]
                    dt = {"f": d_wf, "a": d_wa, "b": d_wb}[kind]
                    nc.sync.dma_start(
                        out=wt[:, off : off + 1024], in_=dt[:, off : off + 1024]
                    )
            nc.scalar.dma_start(out=blob[:, C_WDD:], in_=d_blob[:, C_WDD:])
            nc.gpsimd.dma_start(out=row1, in_=d_row1[:, :])

            st = {}
            stA = {}
            # ---- phase A: extras first, then slot-interleaved over both
            # batch chunks so each weight slab is consumed as it arrives ----
            pex2 = extp.tile([BCH, 2, 4 * NLOC], f32, name="pex2", tag="pex2")
            for bc in range(2):
                pex = pex2[:, bc, :]
                P = big.tile([BCH, NLOC * 128], bf16, name="P", tag="P")
                Q = big.tile([BCH, NLOC * 128], bf16, name="Q", tag="Q")
                stA[bc] = (pex, P, Q)

            for j in range(8):
                c0 = j * 1024
                for bc in range(2):
                    b0 = bc * BCH
                    ctl = blob[:, C_CT + b0 : C_CT + b0 + BCH]
                    cb8 = blob[:, C_CB8 + bc * 1024 : C_CB8 + (bc + 1) * 1024]
                    pex, P, Q = stA[bc]
                    # chol matmul first (S consumes while R matmuls run)
                    spj = sp.tile([BCH, 1024], f32, name="spj", tag="spj")
                    for h in range(2):
                        nc.tensor.matmul(
                            spj[:, h * 512 : (h + 1) * 512],
                            ctl,
                            blob[
                                :, C_CSL + c0 + h * 512 : C_CSL + c0 + (h + 1) * 512
                            ],
                            start=True,
                            stop=True,
                        )
                    rpj = rp.tile([BCH, 1024], f32, name="rpj", tag="rpj")
                    nch = len(EXT_OF_J[j])
                    for t, ext in enumerate(EXT_OF_J[j]):
                        kind, off = SLAB_PACK[(j, t)]
                        wt = wtile[kind]
                        xcol = C_XVT + t * 256 + b0
                        for h in range(2):
                            nc.tensor.matmul(
                                rpj[:, h * 512 : (h + 1) * 512],
                                blob[:ext, xcol : xcol + BCH],
                                wt[:ext, off + h * 512 : off + (h + 1) * 512],
                                start=(t == 0),
                                stop=(t == nch - 1),
                            )
                    # S: square the chol output straight out of PSUM
                    nc.scalar.activation(
                        out=Q[:, c0 : c0 + 1024], in_=spj, func=AF.Square
                    )
                    if j < 6:
                        # S copies R to SBUF; V multiplies at 2x
                        rsj = rsb.tile([BCH, 1024], bf16, name="rsj", tag="rsj")
                        nc.scalar.activation(out=rsj, in_=rpj, func=AF.Copy)
                        nc.vector.tensor_mul(P[:, c0 : c0 + 1024], rsj, cb8)
                    else:
                        # V multiplies straight out of PSUM
                        nc.vector.tensor_mul(P[:, c0 : c0 + 1024], rpj, cb8)

            for bc in range(2):
                b0 = bc * BCH
                ctl = blob[:, C_CT + b0 : C_CT + b0 + BCH]
                pex, P, Q = stA[bc]
                # extras: wamp | bias | 2cg
                nc.tensor.matmul(
                    pex[:, : 3 * NLOC],
                    ctl,
                    blob[:, C_EW : C_EW + 192],
                    start=True,
                    stop=False,
                )
                nc.tensor.matmul(
                    pex[:, : 3 * NLOC],
                    ones_sb,
                    row1[:, 512:704],
                    start=False,
                    stop=True,
                )
                # dotbd into pex[:, 192:256]
                for kc in range(4):
                    xcol = C_XVT + kc * 256 + b0
                    nc.tensor.matmul(
                        pex[:, 3 * NLOC :],
                        blob[:, xcol : xcol + BCH],
                        blob[:, C_BDM + kc * 64 : C_BDM + (kc + 1) * 64],
                        start=(kc == 0),
                        stop=(kc == 3),
                    )
                # add-trees on V, per half (rows 0-31 start after slot 3)
                DOTL = accs.tile([BCH, NLOC], f32, name="DOTL", tag="DOTL")
                SQL = accs.tile([BCH, NLOC], f32, name="SQL", tag="SQL")
                HR = NLOC // 2
                for hh, (src_, dst) in enumerate(
                    ((P, DOTL), (Q, SQL), (P, DOTL), (Q, SQL))
                ):
                    half = hh // 2
                    r0 = half * HR
                    nm = ("d", "s")[hh % 2] + str(half)
                    cur = src_[:, r0 * 128 : (r0 + HR) * 128]
                    w = 128
                    while w > 4:
                        w //= 2
                        nxt = tree.tile(
                            [BCH, HR * w], bf16, name=f"t{nm}{w}", tag=f"t{nm}{w}"
                        )
                        nc.vector.tensor_add(
                            nxt.rearrange("p (r k) -> p r k", k=w),
                            cur.rearrange("p (r k) -> p r k", k=2 * w)[:, :, :w],
                            cur.rearrange("p (r k) -> p r k", k=2 * w)[:, :, w:],
                        )
                        cur = nxt
                    t2 = tree.tile([BCH, HR * 2], f32, name=f"t2{nm}", tag=f"t2{nm}")
                    nc.vector.tensor_add(
                        t2.rearrange("p (r k) -> p r k", k=2),
                        cur.rearrange("p (r k) -> p r k", k=4)[:, :, :2],
                        cur.rearrange("p (r k) -> p r k", k=4)[:, :, 2:],
                    )
                    nc.vector.tensor_add(
                        dst[:, r0 : r0 + HR],
                        t2.rearrange("p (r k) -> p r k", k=2)[:, :, 0],
                        t2.rearrange("p (r k) -> p r k", k=2)[:, :, 1],
                    )
                # diag block (matmul late: PSUM bank freed by slot loop)
                pdg = miscp.tile([BCH, NLOC * FIN], f32, name="pdg", tag="pdg")
                nc.tensor.matmul(
                    pdg, ctl, blob[:, C_WDD : C_WDD + 512], start=True, stop=False
                )
                nc.tensor.matmul(pdg, ones_sb, row1[:, :512], start=False, stop=True)
                xvd = blob[:, C_XVD + bc * 512 : C_XVD + (bc + 1) * 512]
                xle = blob[:, C_XLE + bc * 512 : C_XLE + (bc + 1) * 512]
                expd = tree.tile([BCH, NLOC * FIN], bf16, name="expd", tag="expd")
                nc.scalar.activation(out=expd, in_=pdg, func=AF.Exp)
                sq2 = tree.tile([BCH, NLOC * FIN], bf16, name="sq2", tag="sq2")
                nc.scalar.activation(out=sq2, in_=pdg, func=AF.Exp, scale=2.0)
                SQD = accs.tile([BCH, NLOC], f32, name="SQD", tag="SQD")
                nc.vector.tensor_reduce(
                    out=SQD,
                    in_=sq2.rearrange("p (r f) -> p r f", f=FIN),
                    axis=mybir.AxisListType.X,
                    op=ALU.add,
                )
                prd = tree.tile([BCH, NLOC * FIN], bf16, name="prd", tag="prd")
                nc.vector.tensor_mul(prd, expd, xvd)
                DOTD = accs.tile([BCH, NLOC], f32, name="DOTD", tag="DOTD")
                nc.vector.tensor_reduce(
                    out=DOTD,
                    in_=prd.rearrange("p (r f) -> p r f", f=FIN),
                    axis=mybir.AxisListType.X,
                    op=ALU.add,
                )
                prl = tree.tile([BCH, NLOC * FIN], bf16, name="prl", tag="prl")
                nc.vector.tensor_mul(prl, expd, xle)
                LDS = accs.tile([BCH, NLOC], f32, name="LDS", tag="LDS")
                nc.vector.tensor_reduce(
                    out=LDS,
                    in_=prl.rearrange("p (r f) -> p r f", f=FIN),
                    axis=mybir.AxisListType.X,
                    op=ALU.add,
                )
                st[bc] = dict(
                    pex=pex, pdg=pdg, DOTL=DOTL, SQL=SQL,
                    SQD=SQD, DOTD=DOTD, LDS=LDS,
                )

            # ---- phase C: assembly ----
            for bc in range(2):
                s_ = st[bc]
                sqn = accs.tile([BCH, NLOC], f32, name="sqn", tag="sqn")
                nc.vector.tensor_add(sqn, s_["SQL"], s_["SQD"])
                nc.vector.tensor_add(sqn, sqn, s_["pex"][:, 2 * NLOC : 3 * NLOC])
                dot = accs.tile([BCH, NLOC], f32, name="dot", tag="dot")
                nc.vector.tensor_add(dot, s_["DOTL"], s_["DOTD"])
                nc.vector.tensor_add(dot, dot, s_["pex"][:, 3 * NLOC :])
                s_.update(sqn=sqn, dot=dot)
            for bc in range(2):
                s_ = st[bc]
                l1 = accs.tile([BCH, NLOC], f32, name="l1", tag="l1")
                nc.scalar.activation(out=l1, in_=s_["sqn"], func=AF.Ln)
                l2 = accs.tile([BCH, NLOC], f32, name="l2", tag="l2")
                nc.scalar.activation(out=l2, in_=s_["LDS"], func=AF.Ln)
                s_.update(l1=l1, l2=l2)
            for bc in range(2):
                s_ = st[bc]
                m1 = accs.tile([BCH, NLOC], f32, name="m1", tag="m1")
                nc.scalar.mul(m1, s_["l1"], -0.5)
                u = accs.tile([BCH, NLOC], f32, name="u", tag="u")
                nc.vector.tensor_add(u, s_["pex"][:, :NLOC], m1)
                s_.update(u=u)
            for bc in range(2):
                s_ = st[bc]
                sc = accs.tile([BCH, NLOC], f32, name="sc", tag="sc")
                nc.scalar.activation(out=sc, in_=s_["u"], func=AF.Exp)
                s_.update(sc=sc)
            for bc in range(2):
                b0 = bc * BCH
                s_ = st[bc]
                yv = accs.tile([BCH, NLOC], f32, name="yv", tag="yv")
                nc.vector.tensor_mul(yv, s_["dot"], s_["sc"])
                yb = accs.tile([BCH, NLOC], f32, name="yb", tag="yb")
                nc.vector.tensor_add(yb, yv, s_["pex"][:, NLOC : 2 * NLOC])
                ld = accs.tile([BCH, NLOC], f32, name="ld", tag="ld")
                nc.vector.tensor_add(ld, s_["u"], s_["l2"])
                ob = accs.tile([BCH, NLOC, 2], f32, name="ob", tag="ob")
                nc.gpsimd.tensor_copy(out=ob[:, :, 0], in_=yb)
                nc.gpsimd.tensor_copy(out=ob[:, :, 1], in_=ld)
                nc.sync.dma_start(out=d_out[b0 : b0 + BCH, :, :], in_=ob)

    nc.compile()
    return nc


def _host_prep(x, c, Wd, bd, Wa, ba, Wb, bb):
    """Build the 8 per-core input maps."""
    import ml_dtypes

    bf = ml_dtypes.bfloat16
    x = np.ascontiguousarray(x, dtype=np.float32)
    c = np.ascontiguousarray(c, dtype=np.float32)
    Wd5 = np.ascontiguousarray(Wd, dtype=np.float32).reshape(CDIM, NCH, FOUT, NCH, FIN)
    bd4 = np.ascontiguousarray(bd, dtype=np.float32).reshape(NCH, FOUT, NCH, FIN)
    Wa_ = np.ascontiguousarray(Wa, dtype=np.float32)
    Wb_ = np.ascontiguousarray(Wb, dtype=np.float32)
    ba = np.ascontiguousarray(ba, dtype=np.float32)
    bb = np.ascontiguousarray(bb, dtype=np.float32)

    cT = np.ascontiguousarray(c.T)
    xv = np.ascontiguousarray(x[:, :, 0])
    xl = np.ascontiguousarray(x[:, :, 1])
    xvT = np.ascontiguousarray(xv.T)

    in_maps = []
    for k in range(NCORES):
        chs = _channels(k)
        blob = np.zeros((CDIM, NCOL), dtype=np.float32)
        row1 = np.zeros((1, 704), dtype=np.float32)
        wfp = np.zeros((128, 13 * 1024), dtype=np.float32)
        wap = np.zeros((56, 4 * 1024), dtype=np.float32)
        wbp = np.zeros((120, 4 * 1024), dtype=np.float32)
        packs = {"f": wfp, "a": wap, "b": wbp}

        blob[:, C_CT : C_CT + 256] = cT
        for t in range(4):
            blob[:, C_XVT + t * 256 : C_XVT + (t + 1) * 256] = xvT[
                t * 128 : (t + 1) * 128, :
            ]
        for bc in range(2):
            blob[:, C_CB8 + bc * 1024 : C_CB8 + (bc + 1) * 1024] = np.tile(
                c[bc * BCH : (bc + 1) * BCH, :], (1, 8)
            )

        bdm = np.zeros((I, NLOC), dtype=np.float32)
        for j, ch in enumerate(chs):
            w = ch * FIN
            Gall = Wd5[:, ch, :, :, :].reshape(CDIM, FOUT, I)  # [128, 8, 512]
            for t, ext in enumerate(EXT_OF_J[j]):
                i0 = 128 * t
                n = max(0, min(w - i0, ext))
                if n > 0:
                    kind, off = SLAB_PACK[(j, t)]
                    seg = Gall[:, :, i0 : i0 + n]  # [128, 8, n]
                    packs[kind][:n, off : off + 1024] = seg.transpose(2, 1, 0).reshape(
                        n, 8 * 128
                    )
            for fo in range(FOUT):
                r = j * FOUT + fo
                G = Gall[:, fo, :w]  # [128, w]
                if w > 0:
                    if w <= 128:
                        blob[:, C_CSL + r * 128 : C_CSL + r * 128 + w] = G
                    else:
                        M = G.astype(np.float64) @ G.astype(np.float64).T
                        M += np.eye(CDIM) * (1e-12 * np.trace(M) / CDIM)
                        blob[:, C_CSL + r * 128 : C_CSL + (r + 1) * 128] = (
                            np.linalg.cholesky(M)
                        )
                blob[:, C_WDD + r * FIN : C_WDD + (r + 1) * FIN] = Wd5[:, ch, fo, ch, :]
                row1[0, r * FIN : (r + 1) * FIN] = bd4[ch, fo, ch, :]
                bd_low = bd4[ch, fo, :ch, :].reshape(-1)
                bdm[:w, r] = bd_low
                if w > 0:
                    blob[:, C_EW + 128 + r] = 2.0 * (G @ bd_low)
                    row1[0, 512 + 128 + r] = np.dot(bd_low, bd_low)
            rows = slice(ch * FOUT, (ch + 1) * FOUT)
            blob[:, C_EW + j * FOUT : C_EW + (j + 1) * FOUT] = Wa_[:, rows]
            blob[:, C_EW + 64 + j * FOUT : C_EW + 64 + (j + 1) * FOUT] = Wb_[:, rows]
            row1[0, 512 + j * FOUT : 512 + (j + 1) * FOUT] = ba[rows]
            row1[0, 512 + 64 + j * FOUT : 512 + 64 + (j + 1) * FOUT] = bb[rows]

        for kc in range(4):
            blob[:, C_BDM + kc * 64 : C_BDM + (kc + 1) * 64] = bdm[
                kc * 128 : (kc + 1) * 128, :
            ]

        xvd = np.empty((B, NLOC * FIN), dtype=np.float32)
        xled = np.empty((B, NLOC * FIN), dtype=np.float32)
        for r in range(NLOC):
            ch = chs[r // FOUT]
            xvd[:, r * FIN : (r + 1) * FIN] = xv[:, ch * FIN : (ch + 1) * FIN]
            xled[:, r * FIN : (r + 1) * FIN] = xl[:, ch * FIN : (ch + 1) * FIN]
        np.exp(xled, out=xled)
        for bc in range(2):
            blob[:, C_XVD + bc * 512 : C_XVD + (bc + 1) * 512] = xvd[
                bc * BCH : (bc + 1) * BCH, :
            ]
            blob[:, C_XLE + bc * 512 : C_XLE + (bc + 1) * 512] = xled[
                bc * BCH : (bc + 1) * BCH, :
            ]

        in_maps.append(
            {
                "blob": blob.astype(bf),
                "row1": row1.astype(bf),
                "wf": wfp.astype(bf),
                "wa": wap.astype(bf),
                "wb": wbp.astype(bf),
            }
        )
    return in_maps


def kernel(x, c, Wd, bd, Wa, ba, Wb, bb, _trace=False, _tmpdir=None):
    global _NC
    from concourse.bass_utils import run_bass_kernel_spmd

    if _NC is None:
        _NC = _build_nc()
    in_maps = _host_prep(x, c, Wd, bd, Wa, ba, Wb, bb)
    res = run_bass_kernel_spmd(
        _NC, in_maps, core_ids=list(range(NCORES)), trace=_trace, tmpdir=_tmpdir
    )

    out = np.empty((B, O, 2), dtype=np.float32)
    for k in range(NCORES):
        ok = res.results[k]["out"]
        for j, ch in enumerate(_channels(k)):
            out[:, ch * FOUT : (ch + 1) * FOUT, :] = ok[:, j * FOUT : (j + 1) * FOUT, :]
    if _trace:
        return out, res
    return out
